# revision 22
# baseline (speedup 1.0000x reference)
"""Trainium2 Bass kernel: cached multi-head self-attention decoder block.

Per-core (batch-parallel, B=8 -> 8 cores), fully fused single loop.
v2 design (ACT-exp-bound ~178us target):
  - scores: even/odd head matmuls placed on PE row-halves (tile rows 0:64 /
    64:128) and emitted adjacently -> hardware runs them CONCURRENTLY
  - probs exp'd by ACT directly to fp8e4; PV runs fp8 DoubleRow (2 s-blocks
    per matmul, 2 fp8 MACs/cell/cycle) with the v-cache host-interleaved as
    [pair, eo, ko, 80] fp8 (col 64 = ones -> softmax denominator row 64)
  - per chunk, PV PSUM is immediately staged to SBUF (frees banks for the
    next chunk -> no PE stall, no HAM re-throttle); denominators inverted
    with reciprocal_approx_fast; normalize via f32r ones-broadcast matmul
  - DMA: xT/Wq k-slices interleaved on a dedicated queue so the first
    projection matmul starts ~1us in and chases the DMA
  - output projection: partials over chunks 0..6 computed as chunk-7 PE
    filler, tail only does the last chunk's matmul + fused
    (psum+bias)+partial merge, outT DMA'd per m-slice
"""

import numpy as np
import ml_dtypes
from contextlib import ExitStack

import concourse.bass as bass
import concourse.tile as tile
from concourse import bacc, mybir
from concourse.bass_utils import run_bass_kernel_spmd

F32 = mybir.dt.float32
F32R = mybir.dt.float32r
BF16 = mybir.dt.bfloat16
FP8 = mybir.dt.float8e4
ALU = mybir.AluOpType
ACTF = mybir.ActivationFunctionType
DR = mybir.MatmulPerfMode.DoubleRow
NPBF = ml_dtypes.bfloat16
NPF8 = ml_dtypes.float8_e4m3

B, T, S, NS, NH, HD = 8, 512, 2048, 1024, 16, 64
ST = S + T              # 2560
NC = NS // 128          # 8 chunks (head pairs)
SCN = ST // 128         # 20 s-blocks (16 cache + 4 new)
NPAIR = SCN // 2        # 10 DoubleRow pairs (8 cache + 2 new)
VPW = 2 * 2 * 80        # per-pair bytes/partition in vp (eo x ko x 80)
SCALE2 = float(HD ** -0.5)  # fold both q and k scales into q

LAST_EXEC_NS = None
LAST_RESULTS = None
DEBUG = False


def _emit(ctx, tc, D):
    nc = tc.nc

    # ---------------- constants / persistent SBUF ----------------
    const = ctx.enter_context(tc.tile_pool(name="const", bufs=1))
    bqs_t = const.tile([128, 8], F32, name="bqs_t")
    bop_t = const.tile([128, 8], F32, name="bop_t")
    bvr_t = const.tile([1, NS], BF16, name="bvr_t")
    ones_bf = const.tile([1, 128], BF16, name="ones_bf")
    ones_fr = const.tile([65, 64], F32R, name="ones_fr")
    negc_t = const.tile([128, 1], F32, name="negc_t")
    nc.vector.memset(ones_bf[:], 1.0)
    nc.vector.memset(negc_t[:], -3.0)

    xT_t = const.tile([128, 4096], BF16, name="xT_t")  # k-chunk-major cols

    pers = ctx.enter_context(tc.tile_pool(name="pers", bufs=1))
    qT_t = pers.tile([128, 4096], BF16, name="qT_t")    # (x@Wq+bq)*s2, chunk-major
    knT_t = pers.tile([128, 4096], BF16, name="knT_t")  # (x@Wk)^T, chunk-major
    vnew_t = pers.tile([128, 4096], BF16, name="vnew_t")  # x@Wv+bv natural, tc-major
    outT_t = pers.tile([128, 4096], BF16, name="outT_t")

    wv_pool = ctx.enter_context(tc.tile_pool(name="wv", bufs=1))
    wv_tiles = [wv_pool.tile([128, 512], BF16, name=f"wv_{c}", tag=f"wv{c}")
                for c in range(NC)]
    p1_tiles = [wv_pool.tile([128, 512], BF16, name=f"p1_{m}", tag=f"p1{m}")
                for m in range(8)]

    wpool = ctx.enter_context(tc.tile_pool(name="wts", bufs=1))
    wq_t = [wpool.tile([128, 1024], BF16, name=f"wq{k}", tag=f"wq{k}")
            for k in range(8)]
    wk_t = [wpool.tile([128, 1024], BF16, name=f"wk{k}", tag=f"wk{k}")
            for k in range(8)]
    wvw_t = [wpool.tile([128, 1024], BF16, name=f"wvw{k}", tag=f"wvw{k}")
             for k in range(8)]
    wo_t = [wpool.tile([128, 1024], BF16, name=f"wo{k}", tag=f"wo{k}")
            for k in range(7)]
    wo7hi_t = wpool.tile([64, 1024], BF16, name="wo7hi", tag="wo7hi")
    wo7lo_t = wpool.tile([64, 1024], BF16, name="wo7lo", tag="wo7lo")

    kpool = ctx.enter_context(tc.tile_pool(name="kpair", bufs=2))
    vpool = ctx.enter_context(tc.tile_pool(name="vpair", bufs=2))
    probs = ctx.enter_context(tc.tile_pool(name="probs", bufs=6))
    stpool = ctx.enter_context(tc.tile_pool(name="stage", bufs=2))
    rpool = ctx.enter_context(tc.tile_pool(name="recip", bufs=2))
    tmpool = ctx.enter_context(tc.tile_pool(name="tmo", bufs=2))

    # one 3-deep rotation shared by scores, fillers, normalize and the tail:
    # 6 banks + 2 PV banks = all 8 PSUM banks; 3 bufs give the score matmuls
    # ~1.5 groups of lookahead so exp never waits on a PSUM bank
    spool = ctx.enter_context(tc.tile_pool(name="spsum", bufs=3, space="PSUM"))
    pvpool = ctx.enter_context(tc.tile_pool(name="pvpsum", bufs=1, space="PSUM"))

    def work_tile(name):
        return spool.tile([128, 1024], F32, name=name, tag="sc")[:, 0:512]

    # ---------------- DMA loads ----------------
    # ramp queue (gpsimd): xT first, then weights; q_burst(0)'s k-th matmul
    # chases the k-th Wq slice so the PE starts ~3.5us in
    nc.gpsimd.dma_start(ones_fr[:], D["onesf"].ap()[:, :])  # f32r cast dma
    nc.gpsimd.dma_start(xT_t[:, 0:2048], D["xT"].ap()[:, 0:2048])
    nc.gpsimd.dma_start(xT_t[:, 2048:4096], D["xT"].ap()[:, 2048:4096])
    for k in range(8):
        nc.sync.dma_start(wq_t[k][:], D["Wq"].ap()[k * 128:(k + 1) * 128, :])
    for k in range(8):
        nc.gpsimd.dma_start(wk_t[k][:], D["Wk"].ap()[k * 128:(k + 1) * 128, :])
    for k in range(8):
        nc.gpsimd.dma_start(wvw_t[k][:], D["Wv"].ap()[k * 128:(k + 1) * 128, :])
    for k in range(7):
        nc.gpsimd.dma_start(wo_t[k][:], D["Wo"].ap()[k * 128:(k + 1) * 128, :])
    # chunk-7 Wo halves staged at partitions 0:64 so the tail can consume
    # the odd-half mul output (tm) directly without a partition-shift DMA
    nc.gpsimd.dma_start(wo7hi_t[:], D["Wo"].ap()[896:960, :])
    nc.gpsimd.dma_start(wo7lo_t[:], D["Wo"].ap()[960:1024, :])

    # second queue (sync): small consts + kv-cache chunk prefetches
    nc.sync.dma_start(bqs_t[:], D["bqs"][:, :])

    nc.sync.dma_start(bop_t[:], D["bop"][:, :])
    nc.sync.dma_start(bvr_t[:], D["bvr"][:, :])

    kp_tiles = [None] * NC
    vp_tiles = [None] * NC

    def prefetch(c):
        kp = kpool.tile([128, 2048], BF16, name="kp", tag="kp")
        nc.sync.dma_start(kp[:], D["kTc"].ap()[c * 128:(c + 1) * 128, :])
        vp = vpool.tile([128, 2600], BF16, name="vp", tag="vp")
        nc.sync.dma_start(vp[:, 0:2080],
                          D["vb"].ap()[c * 128:(c + 1) * 128, :])
        # ones slots of the new-token aug blocks
        nc.vector.memset(
            vp[:, 2080:2600].rearrange("p (tc h q) -> p tc h q", h=2, q=65)
            [:, :, :, 64:65], 1.0)
        kp_tiles[c], vp_tiles[c] = kp, vp

    prefetch(0)

    # ---------------- burst helpers (filler PE work) ----------------
    # each burst is split into two 4-matmul halves so a filler never blocks
    # the PE for more than ~0.9us between score groups
    def q_burst(m, half):
        ks = range(4) if half == 0 else range(4, 8)
        if half == 0:
            pt = work_tile(f"qp{m}")
            q_burst.pt[m] = pt
        pt = q_burst.pt[m]
        for k in ks:
            nc.tensor.matmul(pt[:], lhsT=wq_t[k][:, m * 128:(m + 1) * 128],
                             rhs=xT_t[:, k * 512:(k + 1) * 512],
                             start=(k == 0), stop=(k == 7))
        if half == 1:
            nc.vector.tensor_scalar(qT_t[:, m * 512:(m + 1) * 512], pt[:],
                                    SCALE2, bqs_t[:, m:m + 1], ALU.mult, ALU.add)
    q_burst.pt = {}

    def k_burst(c, half):
        ks = range(4) if half == 0 else range(4, 8)
        if half == 0:
            pt = work_tile(f"kp{c}")
            k_burst.pt[c] = pt
        pt = k_burst.pt[c]
        for k in ks:
            nc.tensor.matmul(pt[:], lhsT=wk_t[k][:, c * 128:(c + 1) * 128],
                             rhs=xT_t[:, k * 512:(k + 1) * 512],
                             start=(k == 0), stop=(k == 7))
        if half == 1:
            nc.vector.tensor_copy(knT_t[:, c * 512:(c + 1) * 512], pt[:])
            nc.sync.dma_start(D["keyT"].ap()[c * 128:(c + 1) * 128, :],
                              knT_t[:, c * 512:(c + 1) * 512])
    k_burst.pt = {}

    def v_burst4(cg, t4, half):
        # value[t4-block, chunks 4cg..4cg+3]
        ks = range(4) if half == 0 else range(4, 8)
        if half == 0:
            pt = work_tile(f"vb{cg}{t4}")
            v_burst4.pt[(cg, t4)] = pt
        pt = v_burst4.pt[(cg, t4)]
        for k in ks:
            nc.tensor.matmul(
                pt[:],
                lhsT=xT_t[:, k * 512 + t4 * 128:k * 512 + (t4 + 1) * 128],
                rhs=wvw_t[k][:, cg * 512:(cg + 1) * 512],
                start=(k == 0), stop=False)
        if half == 1:
            nc.tensor.matmul(pt[:], lhsT=ones_bf[0:1, 0:128],
                             rhs=bvr_t[0:1, cg * 512:(cg + 1) * 512],
                             start=False, stop=True)
            nc.vector.tensor_copy(
                vnew_t[:, t4 * 1024 + cg * 512:t4 * 1024 + (cg + 1) * 512],
                pt[:])
    v_burst4.pt = {}

    def vp_aug(c):
        vp = vp_tiles[c]
        for t4 in range(4):
            base = 2080 + t4 * 130
            so = t4 * 1024 + c * 128
            nc.vector.tensor_copy(vp[:, base:base + 64],
                                  vnew_t[:, so:so + 64])
            nc.vector.tensor_copy(vp[:, base + 65:base + 129],
                                  vnew_t[:, so + 64:so + 128])

    def op_a(m):
        # output-projection partial over chunks 0..3 (+ bias), chunk-5/6 filler
        pt = work_tile(f"mpa{m}")
        for cc in range(4):
            nc.tensor.matmul(pt[:], lhsT=wo_t[cc][:, m * 128:(m + 1) * 128],
                             rhs=wv_tiles[cc][:], start=(cc == 0),
                             stop=(cc == 3))
        nc.vector.tensor_scalar(p1_tiles[m][:], pt[:], 1.0,
                                bop_t[:, m:m + 1], ALU.mult, ALU.add)

    def op_b(m):
        # chunks 4..6 partial merged into p1 (chunk-7 filler)
        pt = work_tile(f"mpb{m}")
        for cc in range(4, 7):
            nc.tensor.matmul(pt[:], lhsT=wo_t[cc][:, m * 128:(m + 1) * 128],
                             rhs=wv_tiles[cc][:], start=(cc == 4),
                             stop=(cc == 6))
        nc.vector.tensor_add(p1_tiles[m][:], p1_tiles[m][:], pt[:])

    # normalize split: stage PSUM->SBUF + fast recips at end of chunk c;
    # broadcast matmuls + DVE muls early in chunk c+1 (or tail for c=7)
    chunk_state = {}
    tail_tm = [None]

    def stage_recips(c, pve, pvo):
        pse = stpool.tile([65, 512], F32, name=f"pse{c}", tag="pse")
        pso = stpool.tile([65, 512], F32, name=f"pso{c}", tag="pso")
        # denom rows first: the dma hop to partition 0 (recip_approx_fast
        # mislowers at base_partition 64) + recip overlap the big copies
        nc.vector.tensor_copy(pse[64:65, :], pve[64:65, :])
        nc.vector.tensor_copy(pso[64:65, :], pvo[64:65, :])
        den = rpool.tile([1, 1024], F32, name=f"den{c}", tag="den")
        nc.sync.dma_start(den[0:1, 0:512], pse[64:65, :])
        nc.sync.dma_start(den[0:1, 512:1024], pso[64:65, :])
        nc.vector.tensor_copy(pse[0:64, :], pve[0:64, :])
        nc.vector.tensor_copy(pso[0:64, :], pvo[0:64, :])
        rcp = rpool.tile([1, 1024], F32, name=f"rcp{c}", tag="rcp")
        rcf = rpool.tile([1, 1024], F32R, name=f"rcf{c}", tag="rcf")
        nc.vector.reciprocal_approx_fast(rcp[:], den[:])
        nc.vector.tensor_copy(rcf[:], rcp[:])
        if DEBUG and c == 0:
            nc.sync.dma_start(D["dbg_pse0"].ap()[:, :], pse[:])
            nc.sync.dma_start(D["dbg_pso0"].ap()[:, :], pso[:])
            nc.sync.dma_start(D["dbg_qT"].ap()[:, :], qT_t[:])
        chunk_state[c] = (pse, pso, rcf)

    def normalize(c):
        pse, pso, rcf = chunk_state.pop(c)
        sce = work_tile(f"sce{c}")
        nc.tensor.matmul(sce[0:64, :], lhsT=ones_fr[0:1, :],
                         rhs=rcf[0:1, 0:512], start=True, stop=True)
        sco = work_tile(f"sco{c}")
        nc.tensor.matmul(sco[0:64, :], lhsT=ones_fr[0:1, :],
                         rhs=rcf[0:1, 512:1024], start=True, stop=True)
        if DEBUG and c == 0:
            nc.sync.dma_start(D["dbg_rcp"].ap()[:, :], rcf[0:1, :].bitcast(F32))
            dbg_sce_t = stpool.tile([64, 512], F32, name="dbg_sce_t", tag="dbgs")
            nc.vector.tensor_copy(dbg_sce_t[:], sce[0:64, :])
            nc.sync.dma_start(D["dbg_sce"].ap()[:, :], dbg_sce_t[:])
        nc.vector.tensor_mul(wv_tiles[c][0:64, :], pse[0:64, :], sce[0:64, :])
        tm = tmpool.tile([64, 512], BF16, name=f"tm{c}", tag="tm")
        nc.vector.tensor_mul(tm[:], pso[0:64, :], sco[0:64, :])
        if c == NC - 1:
            tail_tm[0] = tm  # consumed directly by the tail matmuls
        else:
            nc.sync.dma_start(wv_tiles[c][64:128, :], tm[:])

    # ---------------- attention chunk pieces ----------------
    def scores_pair(c, g, kp):
        # 4 matmuls: (se,so) for j=2g then j=2g+1; se on PE rows 0:63 and so
        # on rows 64:127 emitted adjacently -> concurrent row-tiled execution
        se = spool.tile([128, 1024], F32, name="se", tag="sc")
        so = spool.tile([128, 1024], F32, name="so", tag="sc")
        rhs_e = qT_t[0:64, c * 512:(c + 1) * 512]
        rhs_o = qT_t[64:128, c * 512:(c + 1) * 512]
        for jj in range(2):
            j = 2 * g + jj
            if j < 16:
                le = kp[0:64, j * 128:(j + 1) * 128]
                lo = kp[64:128, j * 128:(j + 1) * 128]
            else:
                jo = c * 512 + (j - 16) * 128
                le = knT_t[0:64, jo:jo + 128]
                lo = knT_t[64:128, jo:jo + 128]
            nc.tensor.matmul(se[:, jj * 512:(jj + 1) * 512], lhsT=le,
                             rhs=rhs_e, start=True, stop=True)
            nc.tensor.matmul(so[:, jj * 512:(jj + 1) * 512], lhsT=lo,
                             rhs=rhs_o, start=True, stop=True)
        pe_t = probs.tile([128, 1024], BF16, name="pe", tag="pr")
        nc.scalar.activation(pe_t[:], se[:], ACTF.Exp)
        po_t = probs.tile([128, 1024], BF16, name="po", tag="pr")
        nc.scalar.activation(po_t[:], so[:], ACTF.Exp)
        return pe_t, po_t

    def pv_pair(g, pr, vp, pve, pvo):
        pe_t, po_t = pr
        for jj in range(2):
            j = 2 * g + jj
            nc.tensor.matmul(pve[:], lhsT=vp[:, j * 130:j * 130 + 65],
                             rhs=pe_t[:, jj * 512:(jj + 1) * 512],
                             start=(j == 0), stop=(j == SCN - 1))
            nc.tensor.matmul(pvo[:], lhsT=vp[:, j * 130 + 65:(j + 1) * 130],
                             rhs=po_t[:, jj * 512:(jj + 1) * 512],
                             start=(j == 0), stop=(j == SCN - 1))

    # ---------------- head: q chunk 0 ----------------
    q_burst(0, 0)
    q_burst(0, 1)

    # ---------------- main chunk loop ----------------
    # filler half-burst items per chunk, consumed one per group slot
    def halves(*items):
        out = []
        for it in items:
            if isinstance(it, tuple):
                f, args = it
                out.append(lambda f=f, a=args: f(*a, 0))
                out.append(lambda f=f, a=args: f(*a, 1))
            else:
                out.append(it)
        return out

    chunk_fillers = {
        0: halves((q_burst, (1,)), (k_burst, (0,)), (v_burst4, (0, 0)),
                  (v_burst4, (0, 1)), (v_burst4, (0, 2)), (v_burst4, (0, 3)),
                  lambda: vp_aug(0)),
        1: halves((q_burst, (2,)), (k_burst, (1,)), (v_burst4, (1, 0)),
                  lambda: vp_aug(1)),
        2: halves((q_burst, (3,)), (k_burst, (2,)), (v_burst4, (1, 1)),
                  (v_burst4, (1, 2)), lambda: vp_aug(2)),
        3: halves((q_burst, (4,)), (k_burst, (3,)), (v_burst4, (1, 3)),
                  lambda: vp_aug(3)),
        4: halves((q_burst, (5,)), (k_burst, (4,)), lambda: vp_aug(4)),
        5: halves((q_burst, (6,)), (k_burst, (5,)), lambda: vp_aug(5)) + [
            (lambda m=m: op_a(m)) for m in range(4)],
        6: halves((q_burst, (7,)), (k_burst, (6,)), lambda: vp_aug(6)) + [
            (lambda m=m: op_a(m)) for m in range(4, 8)],
        7: halves((k_burst, (7,)), lambda: vp_aug(7)) + [
            (lambda m=m: op_b(m)) for m in range(8)],
    }

    for c in range(NC):
        kp, vp = kp_tiles[c], vp_tiles[c]
        pr = [None] * NPAIR
        fills = iter(chunk_fillers[c])

        def fill(n=1):
            for _ in range(n):
                f = next(fills, None)
                if f is not None:
                    f()

        # flat software pipeline: the previous chunk's last two PV pairs and
        # its stage/recips are emitted under this chunk's first two score
        # groups, so the PE never drains at a chunk boundary
        for g in range(NPAIR):
            pr[g] = scores_pair(c, g, kp)
            if DEBUG and c == 0 and g == 0:
                nc.sync.dma_start(D["dbg_pe0"].ap()[:, :], pr[0][0][:])
                nc.sync.dma_start(D["dbg_po0"].ap()[:, :], pr[0][1][:])
            if c > 0:
                if g == 0:
                    pv_pair(8, prev_pr[8], prev_vp, prev_pve, prev_pvo)
                elif g == 1:
                    pv_pair(9, prev_pr[9], prev_vp, prev_pve, prev_pvo)
                    stage_recips(c - 1, prev_pve, prev_pvo)
                    if c + 1 < NC:
                        prefetch(c + 1)
            if g == 2:
                pve = pvpool.tile([65, 512], F32, name="pve", tag="pv_e")
                pvo = pvpool.tile([65, 512], F32, name="pvo", tag="pv_o")
            if g >= 2:
                pv_pair(g - 2, pr[g - 2], vp, pve, pvo)
            # normalize(c-1) waits until rcf(c-1) (emitted at g==1) has had
            # ~2 groups of slack so its sce matmul never blocks the PE queue
            if g == 3 and 0 < c < NC - 1:
                normalize(c - 1)
            if g == 2 and c == NC - 1:
                normalize(c - 1)
            if g == 5 and c == 0:
                prefetch(1)
            if c == 0:
                fill(2)
            elif c == NC - 1:
                fill(1 if g < 3 else 2)
            else:
                fill(1)
        fill(20)  # drain any leftovers
        if c == 3:
            nc.sync.dma_start(
                D["value"].ap().rearrange("(tc p) o -> p tc o", p=128),
                vnew_t[:].rearrange("p (tc o) -> p tc o", tc=4))
        prev_pr, prev_vp, prev_pve, prev_pvo = pr, vp, pve, pvo

    # ---------------- tail ----------------
    pv_pair(8, prev_pr[8], prev_vp, prev_pve, prev_pvo)
    pv_pair(9, prev_pr[9], prev_vp, prev_pve, prev_pvo)
    stage_recips(NC - 1, prev_pve, prev_pvo)
    normalize(NC - 1)
    if DEBUG:
        nc.sync.dma_start(D["dbg_wv0"].ap()[:, :], wv_tiles[0][:])
    tm7 = tail_tm[0]
    for m in range(8):
        pt = work_tile(f"op{m}")
        nc.tensor.matmul(pt[:], lhsT=wo7hi_t[:, m * 128:(m + 1) * 128],
                         rhs=wv_tiles[7][0:64, :], start=True, stop=False)
        nc.tensor.matmul(pt[:], lhsT=wo7lo_t[:, m * 128:(m + 1) * 128],
                         rhs=tm7[:], start=False, stop=True)
        # outT = pt + p1   (p1 already carries bias + chunks 0..6)
        nc.vector.tensor_add(outT_t[:, m * 512:(m + 1) * 512], pt[:],
                             p1_tiles[m][:])
        nc.sync.dma_start(
            D["outT"].ap()[m * 128:(m + 1) * 128, :],
            outT_t[:, m * 512:(m + 1) * 512])


def build():
    nc = bacc.Bacc("TRN2", target_bir_lowering=False, debug=False)
    D = {}
    D["xT"] = nc.dram_tensor("xT", [128, 4096], BF16, kind="ExternalInput")
    D["kTc"] = nc.dram_tensor("kTc", [NS, S], BF16, kind="ExternalInput")
    D["vb"] = nc.dram_tensor("vb", [NC * 128, 2080], BF16,
                             kind="ExternalInput")
    for w in ("Wq", "Wk", "Wv", "Wo"):
        D[w] = nc.dram_tensor(w, [NS, NS], BF16, kind="ExternalInput")
    D["bqs"] = nc.dram_tensor("bqs", [128, 8], F32, kind="ExternalInput")
    D["bop"] = nc.dram_tensor("bop", [128, 8], F32, kind="ExternalInput")
    D["bvr"] = nc.dram_tensor("bvr", [1, NS], BF16, kind="ExternalInput")
    D["onesf"] = nc.dram_tensor("onesf", [65, 64], F32, kind="ExternalInput")
    if DEBUG:
        D["dbg_qT"] = nc.dram_tensor("dbg_qT", [128, 4096], BF16, kind="ExternalOutput")
        D["dbg_pe0"] = nc.dram_tensor("dbg_pe0", [128, 1024], BF16, kind="ExternalOutput")
        D["dbg_po0"] = nc.dram_tensor("dbg_po0", [128, 1024], BF16, kind="ExternalOutput")
        D["dbg_pse0"] = nc.dram_tensor("dbg_pse0", [65, 512], F32, kind="ExternalOutput")
        D["dbg_pso0"] = nc.dram_tensor("dbg_pso0", [65, 512], F32, kind="ExternalOutput")
        D["dbg_wv0"] = nc.dram_tensor("dbg_wv0", [128, 512], BF16, kind="ExternalOutput")
        D["dbg_rcp"] = nc.dram_tensor("dbg_rcp", [1, 1024], F32, kind="ExternalOutput")
        D["dbg_sce"] = nc.dram_tensor("dbg_sce", [64, 512], F32, kind="ExternalOutput")
    D["outT"] = nc.dram_tensor("outT", [NS, T], BF16, kind="ExternalOutput")
    D["keyT"] = nc.dram_tensor("keyT", [NS, T], BF16, kind="ExternalOutput")
    D["value"] = nc.dram_tensor("value", [T, NS], BF16, kind="ExternalOutput")

    with tile.TileContext(nc) as tc:
        with ExitStack() as ctx:
            _emit(ctx, tc, D)
    nc.compile()
    return nc


_NC_CACHE = None


def _get_nc():
    global _NC_CACHE
    if _NC_CACHE is None:
        _NC_CACHE = build()
    return _NC_CACHE


def prep_core_inputs(b, x, kv_cache, WqB, WkB, WvB, WoB, bqs, bop, bvr):
    xT = np.ascontiguousarray(x[b].T).reshape(8, 128, 512) \
        .transpose(1, 0, 2).reshape(128, 4096).astype(NPBF)
    kTc = np.ascontiguousarray(kv_cache[b, 0, 0].T).astype(NPBF)  # [NS, S]
    vjp = kv_cache[b, 0, 1].reshape(16, 128, NH, HD)  # [j, p, h, d]
    vh = vjp.transpose(2, 1, 0, 3)                    # [h, p, j, d]
    vb = np.ones((NC, 128, 16, 130), NPBF)
    vb[..., 0:64] = vh[0::2]
    vb[..., 65:129] = vh[1::2]
    return {
        "xT": xT, "kTc": kTc, "vb": vb.reshape(NC * 128, 2080),
        "Wq": WqB, "Wk": WkB, "Wv": WvB, "Wo": WoB,
        "bqs": bqs, "bop": bop, "bvr": bvr,
        "onesf": np.ones((65, 64), np.float32),
    }


def kernel(x, kv_cache, offset=0, Wq=None, bq=None, Wk=None, Wv=None, bv=None,
           Wo=None, bo=None, trace=False):
    global LAST_EXEC_NS, LAST_RESULTS
    x = np.asarray(x, np.float32)
    kv_cache = np.asarray(kv_cache, np.float32)
    Wq, bq, Wk, Wv, bv, Wo, bo = [np.asarray(a, np.float32)
                                  for a in (Wq, bq, Wk, Wv, bv, Wo, bo)]
    WqB, WkB, WvB, WoB = [w.astype(NPBF) for w in (Wq, Wk, Wv, Wo)]
    bqs = np.ascontiguousarray((bq * SCALE2).reshape(8, 128).T)
    bop = np.ascontiguousarray(bo.reshape(8, 128).T)
    bvr = bv[None, :].astype(NPBF)
    in_maps = [prep_core_inputs(b, x, kv_cache, WqB, WkB, WvB, WoB,
                                bqs, bop, bvr) for b in range(B)]
    nc = _get_nc()
    res = run_bass_kernel_spmd(nc, in_maps, core_ids=list(range(B)), trace=trace)
    LAST_EXEC_NS = res.exec_time_ns
    LAST_RESULTS = res
    out = np.stack([res.results[b]["outT"].astype(np.float32).T
                    for b in range(B)])
    key = np.stack([res.results[b]["keyT"].astype(np.float32).T
                    for b in range(B)])
    value = np.stack([res.results[b]["value"].astype(np.float32)
                      for b in range(B)])
    return (np.ascontiguousarray(out), np.ascontiguousarray(key),
            np.ascontiguousarray(value))


# revision 23
# speedup vs baseline: 1.0094x; 1.0094x over previous
"""Trainium2 Bass kernel: cached multi-head self-attention decoder block.

Per-core (batch-parallel, B=8 -> 8 cores), fully fused single loop.
v2 design (ACT-exp-bound ~178us target):
  - scores: even/odd head matmuls placed on PE row-halves (tile rows 0:64 /
    64:128) and emitted adjacently -> hardware runs them CONCURRENTLY
  - probs exp'd by ACT directly to fp8e4; PV runs fp8 DoubleRow (2 s-blocks
    per matmul, 2 fp8 MACs/cell/cycle) with the v-cache host-interleaved as
    [pair, eo, ko, 80] fp8 (col 64 = ones -> softmax denominator row 64)
  - per chunk, PV PSUM is immediately staged to SBUF (frees banks for the
    next chunk -> no PE stall, no HAM re-throttle); denominators inverted
    with reciprocal_approx_fast; normalize via f32r ones-broadcast matmul
  - DMA: xT/Wq k-slices interleaved on a dedicated queue so the first
    projection matmul starts ~1us in and chases the DMA
  - output projection: partials over chunks 0..6 computed as chunk-7 PE
    filler, tail only does the last chunk's matmul + fused
    (psum+bias)+partial merge, outT DMA'd per m-slice
"""

import numpy as np
import ml_dtypes
from contextlib import ExitStack

import concourse.bass as bass
import concourse.tile as tile
from concourse import bacc, mybir
from concourse.bass_utils import run_bass_kernel_spmd

F32 = mybir.dt.float32
F32R = mybir.dt.float32r
BF16 = mybir.dt.bfloat16
FP8 = mybir.dt.float8e4
ALU = mybir.AluOpType
ACTF = mybir.ActivationFunctionType
DR = mybir.MatmulPerfMode.DoubleRow
NPBF = ml_dtypes.bfloat16
NPF8 = ml_dtypes.float8_e4m3

B, T, S, NS, NH, HD = 8, 512, 2048, 1024, 16, 64
ST = S + T              # 2560
NC = NS // 128          # 8 chunks (head pairs)
SCN = ST // 128         # 20 s-blocks (16 cache + 4 new)
NPAIR = SCN // 2        # 10 DoubleRow pairs (8 cache + 2 new)
VPW = 2 * 2 * 80        # per-pair bytes/partition in vp (eo x ko x 80)
SCALE2 = float(HD ** -0.5)  # fold both q and k scales into q

LAST_EXEC_NS = None
LAST_RESULTS = None
DEBUG = False


def _emit(ctx, tc, D):
    nc = tc.nc

    # ---------------- constants / persistent SBUF ----------------
    const = ctx.enter_context(tc.tile_pool(name="const", bufs=1))
    bqs_t = const.tile([128, 8], F32, name="bqs_t")
    bop_t = const.tile([128, 8], F32, name="bop_t")
    bvr_t = const.tile([1, NS], BF16, name="bvr_t")
    ones_bf = const.tile([1, 128], BF16, name="ones_bf")
    ones_fr = const.tile([65, 64], F32R, name="ones_fr")
    negc_t = const.tile([128, 1], F32, name="negc_t")
    nc.vector.memset(ones_bf[:], 1.0)
    nc.vector.memset(negc_t[:], -3.0)

    xT_t = const.tile([128, 4096], BF16, name="xT_t")  # k-chunk-major cols

    pers = ctx.enter_context(tc.tile_pool(name="pers", bufs=1))
    qT_t = pers.tile([128, 4096], BF16, name="qT_t")    # (x@Wq+bq)*s2, chunk-major
    knT_t = pers.tile([128, 4096], BF16, name="knT_t")  # (x@Wk)^T, chunk-major
    vnew_t = pers.tile([128, 4096], BF16, name="vnew_t")  # x@Wv+bv natural, tc-major
    outT_t = pers.tile([128, 4096], BF16, name="outT_t")

    wv_pool = ctx.enter_context(tc.tile_pool(name="wv", bufs=1))
    wv_tiles = [wv_pool.tile([128, 512], BF16, name=f"wv_{c}", tag=f"wv{c}")
                for c in range(NC)]
    p1_tiles = [wv_pool.tile([128, 512], BF16, name=f"p1_{m}", tag=f"p1{m}")
                for m in range(8)]

    wpool = ctx.enter_context(tc.tile_pool(name="wts", bufs=1))
    wq_t = [wpool.tile([128, 1024], BF16, name=f"wq{k}", tag=f"wq{k}")
            for k in range(8)]
    wk_t = [wpool.tile([128, 1024], BF16, name=f"wk{k}", tag=f"wk{k}")
            for k in range(8)]
    wvw_t = [wpool.tile([128, 1024], BF16, name=f"wvw{k}", tag=f"wvw{k}")
             for k in range(8)]
    wo_t = [wpool.tile([128, 1024], BF16, name=f"wo{k}", tag=f"wo{k}")
            for k in range(7)]
    wo7hi_t = wpool.tile([64, 1024], BF16, name="wo7hi", tag="wo7hi")
    wo7lo_t = wpool.tile([64, 1024], BF16, name="wo7lo", tag="wo7lo")

    kpool = ctx.enter_context(tc.tile_pool(name="kpair", bufs=2))
    vpool = ctx.enter_context(tc.tile_pool(name="vpair", bufs=2))
    probs = ctx.enter_context(tc.tile_pool(name="probs", bufs=10))
    stpool = ctx.enter_context(tc.tile_pool(name="stage", bufs=2))
    rpool = ctx.enter_context(tc.tile_pool(name="recip", bufs=2))
    tmpool = ctx.enter_context(tc.tile_pool(name="tmo", bufs=2))

    # one 3-deep rotation shared by scores, fillers, normalize and the tail:
    # 6 banks + 2 PV banks = all 8 PSUM banks; 3 bufs give the score matmuls
    # ~1.5 groups of lookahead so exp never waits on a PSUM bank
    spool = ctx.enter_context(tc.tile_pool(name="spsum", bufs=3, space="PSUM"))
    pvpool = ctx.enter_context(tc.tile_pool(name="pvpsum", bufs=1, space="PSUM"))

    def work_tile(name):
        return spool.tile([128, 1024], F32, name=name, tag="sc")[:, 0:512]

    # ---------------- DMA loads ----------------
    # ramp queue (gpsimd): xT first, then weights; q_burst(0)'s k-th matmul
    # chases the k-th Wq slice so the PE starts ~3.5us in
    nc.gpsimd.dma_start(ones_fr[:], D["onesf"].ap()[:, :])  # f32r cast dma
    nc.gpsimd.dma_start(xT_t[:, 0:2048], D["xT"].ap()[:, 0:2048])
    nc.gpsimd.dma_start(xT_t[:, 2048:4096], D["xT"].ap()[:, 2048:4096])
    for k in range(8):
        nc.sync.dma_start(wq_t[k][:], D["Wq"].ap()[k * 128:(k + 1) * 128, :])
    for k in range(8):
        nc.gpsimd.dma_start(wk_t[k][:], D["Wk"].ap()[k * 128:(k + 1) * 128, :])
    for k in range(8):
        nc.gpsimd.dma_start(wvw_t[k][:], D["Wv"].ap()[k * 128:(k + 1) * 128, :])
    for k in range(7):
        nc.gpsimd.dma_start(wo_t[k][:], D["Wo"].ap()[k * 128:(k + 1) * 128, :])
    # chunk-7 Wo halves staged at partitions 0:64 so the tail can consume
    # the odd-half mul output (tm) directly without a partition-shift DMA
    nc.gpsimd.dma_start(wo7hi_t[:], D["Wo"].ap()[896:960, :])
    nc.gpsimd.dma_start(wo7lo_t[:], D["Wo"].ap()[960:1024, :])

    # second queue (sync): small consts + kv-cache chunk prefetches
    nc.sync.dma_start(bqs_t[:], D["bqs"][:, :])

    nc.sync.dma_start(bop_t[:], D["bop"][:, :])
    nc.sync.dma_start(bvr_t[:], D["bvr"][:, :])

    kp_tiles = [None] * NC
    vp_tiles = [None] * NC

    def prefetch(c):
        kp = kpool.tile([128, 2048], BF16, name="kp", tag="kp")
        nc.sync.dma_start(kp[:], D["kTc"].ap()[c * 128:(c + 1) * 128, :])
        vp = vpool.tile([128, 2600], BF16, name="vp", tag="vp")
        nc.sync.dma_start(vp[:, 0:2080],
                          D["vb"].ap()[c * 128:(c + 1) * 128, :])
        # ones slots of the new-token aug blocks
        nc.vector.memset(
            vp[:, 2080:2600].rearrange("p (tc h q) -> p tc h q", h=2, q=65)
            [:, :, :, 64:65], 1.0)
        kp_tiles[c], vp_tiles[c] = kp, vp

    prefetch(0)

    # ---------------- burst helpers (filler PE work) ----------------
    # each burst is split into two 4-matmul halves so a filler never blocks
    # the PE for more than ~0.9us between score groups
    def q_burst(m, half):
        ks = range(4) if half == 0 else range(4, 8)
        if half == 0:
            pt = work_tile(f"qp{m}")
            q_burst.pt[m] = pt
        pt = q_burst.pt[m]
        for k in ks:
            nc.tensor.matmul(pt[:], lhsT=wq_t[k][:, m * 128:(m + 1) * 128],
                             rhs=xT_t[:, k * 512:(k + 1) * 512],
                             start=(k == 0), stop=(k == 7))
        if half == 1:
            nc.vector.tensor_scalar(qT_t[:, m * 512:(m + 1) * 512], pt[:],
                                    SCALE2, bqs_t[:, m:m + 1], ALU.mult, ALU.add)
    q_burst.pt = {}

    def k_burst(c, half):
        ks = range(4) if half == 0 else range(4, 8)
        if half == 0:
            pt = work_tile(f"kp{c}")
            k_burst.pt[c] = pt
        pt = k_burst.pt[c]
        for k in ks:
            nc.tensor.matmul(pt[:], lhsT=wk_t[k][:, c * 128:(c + 1) * 128],
                             rhs=xT_t[:, k * 512:(k + 1) * 512],
                             start=(k == 0), stop=(k == 7))
        if half == 1:
            nc.vector.tensor_copy(knT_t[:, c * 512:(c + 1) * 512], pt[:])
            nc.sync.dma_start(D["keyT"].ap()[c * 128:(c + 1) * 128, :],
                              knT_t[:, c * 512:(c + 1) * 512])
    k_burst.pt = {}

    def v_burst4(cg, t4, half):
        # value[t4-block, chunks 4cg..4cg+3]
        ks = range(4) if half == 0 else range(4, 8)
        if half == 0:
            pt = work_tile(f"vb{cg}{t4}")
            v_burst4.pt[(cg, t4)] = pt
        pt = v_burst4.pt[(cg, t4)]
        for k in ks:
            nc.tensor.matmul(
                pt[:],
                lhsT=xT_t[:, k * 512 + t4 * 128:k * 512 + (t4 + 1) * 128],
                rhs=wvw_t[k][:, cg * 512:(cg + 1) * 512],
                start=(k == 0), stop=False)
        if half == 1:
            nc.tensor.matmul(pt[:], lhsT=ones_bf[0:1, 0:128],
                             rhs=bvr_t[0:1, cg * 512:(cg + 1) * 512],
                             start=False, stop=True)
            nc.vector.tensor_copy(
                vnew_t[:, t4 * 1024 + cg * 512:t4 * 1024 + (cg + 1) * 512],
                pt[:])
    v_burst4.pt = {}

    def vp_aug(c):
        vp = vp_tiles[c]
        for t4 in range(4):
            base = 2080 + t4 * 130
            so = t4 * 1024 + c * 128
            nc.vector.tensor_copy(vp[:, base:base + 64],
                                  vnew_t[:, so:so + 64])
            nc.vector.tensor_copy(vp[:, base + 65:base + 129],
                                  vnew_t[:, so + 64:so + 128])

    def op_a(m):
        # output-projection partial over chunks 0..3 (+ bias), chunk-5/6 filler
        pt = work_tile(f"mpa{m}")
        for cc in range(4):
            nc.tensor.matmul(pt[:], lhsT=wo_t[cc][:, m * 128:(m + 1) * 128],
                             rhs=wv_tiles[cc][:], start=(cc == 0),
                             stop=(cc == 3))
        nc.vector.tensor_scalar(p1_tiles[m][:], pt[:], 1.0,
                                bop_t[:, m:m + 1], ALU.mult, ALU.add)

    def op_b(m):
        # chunks 4..6 partial merged into p1 (chunk-7 filler)
        pt = work_tile(f"mpb{m}")
        for cc in range(4, 7):
            nc.tensor.matmul(pt[:], lhsT=wo_t[cc][:, m * 128:(m + 1) * 128],
                             rhs=wv_tiles[cc][:], start=(cc == 4),
                             stop=(cc == 6))
        nc.vector.tensor_add(p1_tiles[m][:], p1_tiles[m][:], pt[:])

    # normalize split: stage PSUM->SBUF + fast recips at end of chunk c;
    # broadcast matmuls + DVE muls early in chunk c+1 (or tail for c=7)
    chunk_state = {}
    tail_tm = [None]

    def stage_recips(c, pve, pvo):
        pse = stpool.tile([65, 512], F32, name=f"pse{c}", tag="pse")
        pso = stpool.tile([65, 512], F32, name=f"pso{c}", tag="pso")
        # denom rows first: the dma hop to partition 0 (recip_approx_fast
        # mislowers at base_partition 64) + recip overlap the big copies
        nc.vector.tensor_copy(pse[64:65, :], pve[64:65, :])
        nc.vector.tensor_copy(pso[64:65, :], pvo[64:65, :])
        den = rpool.tile([1, 1024], F32, name=f"den{c}", tag="den")
        nc.sync.dma_start(den[0:1, 0:512], pse[64:65, :])
        nc.sync.dma_start(den[0:1, 512:1024], pso[64:65, :])
        nc.vector.tensor_copy(pse[0:64, :], pve[0:64, :])
        nc.vector.tensor_copy(pso[0:64, :], pvo[0:64, :])
        rcp = rpool.tile([1, 1024], F32, name=f"rcp{c}", tag="rcp")
        rcf = rpool.tile([1, 1024], F32R, name=f"rcf{c}", tag="rcf")
        nc.vector.reciprocal_approx_fast(rcp[:], den[:])
        nc.vector.tensor_copy(rcf[:], rcp[:])
        if DEBUG and c == 0:
            nc.sync.dma_start(D["dbg_pse0"].ap()[:, :], pse[:])
            nc.sync.dma_start(D["dbg_pso0"].ap()[:, :], pso[:])
            nc.sync.dma_start(D["dbg_qT"].ap()[:, :], qT_t[:])
        chunk_state[c] = (pse, pso, rcf)

    def normalize(c):
        pse, pso, rcf = chunk_state.pop(c)
        sce = work_tile(f"sce{c}")
        nc.tensor.matmul(sce[0:64, :], lhsT=ones_fr[0:1, :],
                         rhs=rcf[0:1, 0:512], start=True, stop=True)
        sco = work_tile(f"sco{c}")
        nc.tensor.matmul(sco[0:64, :], lhsT=ones_fr[0:1, :],
                         rhs=rcf[0:1, 512:1024], start=True, stop=True)
        if DEBUG and c == 0:
            nc.sync.dma_start(D["dbg_rcp"].ap()[:, :], rcf[0:1, :].bitcast(F32))
            dbg_sce_t = stpool.tile([64, 512], F32, name="dbg_sce_t", tag="dbgs")
            nc.vector.tensor_copy(dbg_sce_t[:], sce[0:64, :])
            nc.sync.dma_start(D["dbg_sce"].ap()[:, :], dbg_sce_t[:])
        nc.vector.tensor_mul(wv_tiles[c][0:64, :], pse[0:64, :], sce[0:64, :])
        tm = tmpool.tile([64, 512], BF16, name=f"tm{c}", tag="tm")
        nc.vector.tensor_mul(tm[:], pso[0:64, :], sco[0:64, :])
        if c == NC - 1:
            tail_tm[0] = tm  # consumed directly by the tail matmuls
        else:
            nc.sync.dma_start(wv_tiles[c][64:128, :], tm[:])

    # ---------------- attention chunk pieces ----------------
    def scores_pair(c, g, kp):
        # 4 matmuls: (se,so) for j=2g then j=2g+1; se on PE rows 0:63 and so
        # on rows 64:127 emitted adjacently -> concurrent row-tiled execution
        se = spool.tile([128, 1024], F32, name="se", tag="sc")
        so = spool.tile([128, 1024], F32, name="so", tag="sc")
        rhs_e = qT_t[0:64, c * 512:(c + 1) * 512]
        rhs_o = qT_t[64:128, c * 512:(c + 1) * 512]
        for jj in range(2):
            j = 2 * g + jj
            if j < 16:
                le = kp[0:64, j * 128:(j + 1) * 128]
                lo = kp[64:128, j * 128:(j + 1) * 128]
            else:
                jo = c * 512 + (j - 16) * 128
                le = knT_t[0:64, jo:jo + 128]
                lo = knT_t[64:128, jo:jo + 128]
            nc.tensor.matmul(se[:, jj * 512:(jj + 1) * 512], lhsT=le,
                             rhs=rhs_e, start=True, stop=True)
            nc.tensor.matmul(so[:, jj * 512:(jj + 1) * 512], lhsT=lo,
                             rhs=rhs_o, start=True, stop=True)
        pe_t = probs.tile([128, 1024], BF16, name="pe", tag="pr")
        nc.scalar.activation(pe_t[:], se[:], ACTF.Exp)
        po_t = probs.tile([128, 1024], BF16, name="po", tag="pr")
        nc.scalar.activation(po_t[:], so[:], ACTF.Exp)
        return pe_t, po_t

    def pv_pair(g, pr, vp, pve, pvo):
        pe_t, po_t = pr
        for jj in range(2):
            j = 2 * g + jj
            nc.tensor.matmul(pve[:], lhsT=vp[:, j * 130:j * 130 + 65],
                             rhs=pe_t[:, jj * 512:(jj + 1) * 512],
                             start=(j == 0), stop=(j == SCN - 1))
            nc.tensor.matmul(pvo[:], lhsT=vp[:, j * 130 + 65:(j + 1) * 130],
                             rhs=po_t[:, jj * 512:(jj + 1) * 512],
                             start=(j == 0), stop=(j == SCN - 1))

    # ---------------- head: q chunk 0 ----------------
    q_burst(0, 0)
    q_burst(0, 1)

    # ---------------- main chunk loop ----------------
    # filler half-burst items per chunk, consumed one per group slot
    def halves(*items):
        out = []
        for it in items:
            if isinstance(it, tuple):
                f, args = it
                out.append(lambda f=f, a=args: f(*a, 0))
                out.append(lambda f=f, a=args: f(*a, 1))
            else:
                out.append(it)
        return out

    chunk_fillers = {
        0: halves((q_burst, (1,)), (k_burst, (0,)), (v_burst4, (0, 0)),
                  (v_burst4, (0, 1)), (v_burst4, (0, 2)), (v_burst4, (0, 3)),
                  lambda: vp_aug(0)),
        1: halves((q_burst, (2,)), (k_burst, (1,)), (v_burst4, (1, 0)),
                  lambda: vp_aug(1)),
        2: halves((q_burst, (3,)), (k_burst, (2,)), (v_burst4, (1, 1)),
                  (v_burst4, (1, 2)), lambda: vp_aug(2)),
        3: halves((q_burst, (4,)), (k_burst, (3,)), (v_burst4, (1, 3)),
                  lambda: vp_aug(3)),
        4: halves((q_burst, (5,)), (k_burst, (4,)), lambda: vp_aug(4)),
        5: halves((q_burst, (6,)), (k_burst, (5,)), lambda: vp_aug(5)) + [
            (lambda m=m: op_a(m)) for m in range(4)],
        6: halves((q_burst, (7,)), (k_burst, (6,)), lambda: vp_aug(6)) + [
            (lambda m=m: op_a(m)) for m in range(4, 8)],
        7: halves((k_burst, (7,)), lambda: vp_aug(7)) + [
            (lambda m=m: op_b(m)) for m in range(8)],
    }

    for c in range(NC):
        kp, vp = kp_tiles[c], vp_tiles[c]
        pr = [None] * NPAIR
        fills = iter(chunk_fillers[c])

        def fill(n=1):
            for _ in range(n):
                f = next(fills, None)
                if f is not None:
                    f()

        # flat software pipeline with a 4-group PV lag: a PV matmul only
        # enters the strict-FIFO PE queue when its exp is long done, so the
        # queue never parks; the previous chunk's pv(6..9) ride under this
        # chunk's first four score groups
        for g in range(NPAIR):
            pr[g] = scores_pair(c, g, kp)
            if DEBUG and c == 0 and g == 0:
                nc.sync.dma_start(D["dbg_pe0"].ap()[:, :], pr[0][0][:])
                nc.sync.dma_start(D["dbg_po0"].ap()[:, :], pr[0][1][:])
            if c > 0 and g < 4:
                pv_pair(6 + g, prev_pr[6 + g], prev_vp, prev_pve, prev_pvo)
                if g == 3:
                    stage_recips(c - 1, prev_pve, prev_pvo)
                    if c + 1 < NC:
                        prefetch(c + 1)
            if g == 4:
                pve = pvpool.tile([65, 512], F32, name="pve", tag="pv_e")
                pvo = pvpool.tile([65, 512], F32, name="pvo", tag="pv_o")
            if g >= 4:
                pv_pair(g - 4, pr[g - 4], vp, pve, pvo)
            # normalize(c-1) after rcf(c-1) (emitted g==3) has ~2 groups of
            # slack so its sce matmul never blocks the PE queue; chunk 7's
            # op_b fillers (first consumed at g==5) need wv up to 6 first
            if g == 5 and c > 0:
                normalize(c - 1)
            if g == 5 and c == 0:
                prefetch(1)
            if c == 0:
                fill(2)
            elif c == NC - 1:
                fill((1, 1, 1, 0, 0, 2, 2, 2, 2, 2)[g])
            else:
                fill(1)
        fill(20)  # drain any leftovers
        if c == 3:
            nc.sync.dma_start(
                D["value"].ap().rearrange("(tc p) o -> p tc o", p=128),
                vnew_t[:].rearrange("p (tc o) -> p tc o", tc=4))
        prev_pr, prev_vp, prev_pve, prev_pvo = pr, vp, pve, pvo

    # ---------------- tail ----------------
    for g in range(4):
        pv_pair(6 + g, prev_pr[6 + g], prev_vp, prev_pve, prev_pvo)
    stage_recips(NC - 1, prev_pve, prev_pvo)
    normalize(NC - 1)
    if DEBUG:
        nc.sync.dma_start(D["dbg_wv0"].ap()[:, :], wv_tiles[0][:])
    tm7 = tail_tm[0]
    for m in range(8):
        pt = work_tile(f"op{m}")
        nc.tensor.matmul(pt[:], lhsT=wo7hi_t[:, m * 128:(m + 1) * 128],
                         rhs=wv_tiles[7][0:64, :], start=True, stop=False)
        nc.tensor.matmul(pt[:], lhsT=wo7lo_t[:, m * 128:(m + 1) * 128],
                         rhs=tm7[:], start=False, stop=True)
        # outT = pt + p1   (p1 already carries bias + chunks 0..6)
        nc.vector.tensor_add(outT_t[:, m * 512:(m + 1) * 512], pt[:],
                             p1_tiles[m][:])
        nc.sync.dma_start(
            D["outT"].ap()[m * 128:(m + 1) * 128, :],
            outT_t[:, m * 512:(m + 1) * 512])


def build():
    nc = bacc.Bacc("TRN2", target_bir_lowering=False, debug=False)
    D = {}
    D["xT"] = nc.dram_tensor("xT", [128, 4096], BF16, kind="ExternalInput")
    D["kTc"] = nc.dram_tensor("kTc", [NS, S], BF16, kind="ExternalInput")
    D["vb"] = nc.dram_tensor("vb", [NC * 128, 2080], BF16,
                             kind="ExternalInput")
    for w in ("Wq", "Wk", "Wv", "Wo"):
        D[w] = nc.dram_tensor(w, [NS, NS], BF16, kind="ExternalInput")
    D["bqs"] = nc.dram_tensor("bqs", [128, 8], F32, kind="ExternalInput")
    D["bop"] = nc.dram_tensor("bop", [128, 8], F32, kind="ExternalInput")
    D["bvr"] = nc.dram_tensor("bvr", [1, NS], BF16, kind="ExternalInput")
    D["onesf"] = nc.dram_tensor("onesf", [65, 64], F32, kind="ExternalInput")
    if DEBUG:
        D["dbg_qT"] = nc.dram_tensor("dbg_qT", [128, 4096], BF16, kind="ExternalOutput")
        D["dbg_pe0"] = nc.dram_tensor("dbg_pe0", [128, 1024], BF16, kind="ExternalOutput")
        D["dbg_po0"] = nc.dram_tensor("dbg_po0", [128, 1024], BF16, kind="ExternalOutput")
        D["dbg_pse0"] = nc.dram_tensor("dbg_pse0", [65, 512], F32, kind="ExternalOutput")
        D["dbg_pso0"] = nc.dram_tensor("dbg_pso0", [65, 512], F32, kind="ExternalOutput")
        D["dbg_wv0"] = nc.dram_tensor("dbg_wv0", [128, 512], BF16, kind="ExternalOutput")
        D["dbg_rcp"] = nc.dram_tensor("dbg_rcp", [1, 1024], F32, kind="ExternalOutput")
        D["dbg_sce"] = nc.dram_tensor("dbg_sce", [64, 512], F32, kind="ExternalOutput")
    D["outT"] = nc.dram_tensor("outT", [NS, T], BF16, kind="ExternalOutput")
    D["keyT"] = nc.dram_tensor("keyT", [NS, T], BF16, kind="ExternalOutput")
    D["value"] = nc.dram_tensor("value", [T, NS], BF16, kind="ExternalOutput")

    with tile.TileContext(nc) as tc:
        with ExitStack() as ctx:
            _emit(ctx, tc, D)
    nc.compile()
    return nc


_NC_CACHE = None


def _get_nc():
    global _NC_CACHE
    if _NC_CACHE is None:
        _NC_CACHE = build()
    return _NC_CACHE


def prep_core_inputs(b, x, kv_cache, WqB, WkB, WvB, WoB, bqs, bop, bvr):
    xT = np.ascontiguousarray(x[b].T).reshape(8, 128, 512) \
        .transpose(1, 0, 2).reshape(128, 4096).astype(NPBF)
    kTc = np.ascontiguousarray(kv_cache[b, 0, 0].T).astype(NPBF)  # [NS, S]
    vjp = kv_cache[b, 0, 1].reshape(16, 128, NH, HD)  # [j, p, h, d]
    vh = vjp.transpose(2, 1, 0, 3)                    # [h, p, j, d]
    vb = np.ones((NC, 128, 16, 130), NPBF)
    vb[..., 0:64] = vh[0::2]
    vb[..., 65:129] = vh[1::2]
    return {
        "xT": xT, "kTc": kTc, "vb": vb.reshape(NC * 128, 2080),
        "Wq": WqB, "Wk": WkB, "Wv": WvB, "Wo": WoB,
        "bqs": bqs, "bop": bop, "bvr": bvr,
        "onesf": np.ones((65, 64), np.float32),
    }


def kernel(x, kv_cache, offset=0, Wq=None, bq=None, Wk=None, Wv=None, bv=None,
           Wo=None, bo=None, trace=False):
    global LAST_EXEC_NS, LAST_RESULTS
    x = np.asarray(x, np.float32)
    kv_cache = np.asarray(kv_cache, np.float32)
    Wq, bq, Wk, Wv, bv, Wo, bo = [np.asarray(a, np.float32)
                                  for a in (Wq, bq, Wk, Wv, bv, Wo, bo)]
    WqB, WkB, WvB, WoB = [w.astype(NPBF) for w in (Wq, Wk, Wv, Wo)]
    bqs = np.ascontiguousarray((bq * SCALE2).reshape(8, 128).T)
    bop = np.ascontiguousarray(bo.reshape(8, 128).T)
    bvr = bv[None, :].astype(NPBF)
    in_maps = [prep_core_inputs(b, x, kv_cache, WqB, WkB, WvB, WoB,
                                bqs, bop, bvr) for b in range(B)]
    nc = _get_nc()
    res = run_bass_kernel_spmd(nc, in_maps, core_ids=list(range(B)), trace=trace)
    LAST_EXEC_NS = res.exec_time_ns
    LAST_RESULTS = res
    out = np.stack([res.results[b]["outT"].astype(np.float32).T
                    for b in range(B)])
    key = np.stack([res.results[b]["keyT"].astype(np.float32).T
                    for b in range(B)])
    value = np.stack([res.results[b]["value"].astype(np.float32)
                      for b in range(B)])
    return (np.ascontiguousarray(out), np.ascontiguousarray(key),
            np.ascontiguousarray(value))


# revision 24
# speedup vs baseline: 1.0254x; 1.0158x over previous
"""Trainium2 Bass kernel: cached multi-head self-attention decoder block.

Per-core (batch-parallel, B=8 -> 8 cores), fully fused single loop.
v2 design (ACT-exp-bound ~178us target):
  - scores: even/odd head matmuls placed on PE row-halves (tile rows 0:64 /
    64:128) and emitted adjacently -> hardware runs them CONCURRENTLY
  - probs exp'd by ACT directly to fp8e4; PV runs fp8 DoubleRow (2 s-blocks
    per matmul, 2 fp8 MACs/cell/cycle) with the v-cache host-interleaved as
    [pair, eo, ko, 80] fp8 (col 64 = ones -> softmax denominator row 64)
  - per chunk, PV PSUM is immediately staged to SBUF (frees banks for the
    next chunk -> no PE stall, no HAM re-throttle); denominators inverted
    with reciprocal_approx_fast; normalize via f32r ones-broadcast matmul
  - DMA: xT/Wq k-slices interleaved on a dedicated queue so the first
    projection matmul starts ~1us in and chases the DMA
  - output projection: partials over chunks 0..6 computed as chunk-7 PE
    filler, tail only does the last chunk's matmul + fused
    (psum+bias)+partial merge, outT DMA'd per m-slice
"""

import numpy as np
import ml_dtypes
from contextlib import ExitStack

import concourse.bass as bass
import concourse.tile as tile
from concourse import bacc, mybir
from concourse.bass_utils import run_bass_kernel_spmd

F32 = mybir.dt.float32
F32R = mybir.dt.float32r
BF16 = mybir.dt.bfloat16
FP8 = mybir.dt.float8e4
ALU = mybir.AluOpType
ACTF = mybir.ActivationFunctionType
DR = mybir.MatmulPerfMode.DoubleRow
NPBF = ml_dtypes.bfloat16
NPF8 = ml_dtypes.float8_e4m3

B, T, S, NS, NH, HD = 8, 512, 2048, 1024, 16, 64
ST = S + T              # 2560
NC = NS // 128          # 8 chunks (head pairs)
SCN = ST // 128         # 20 s-blocks (16 cache + 4 new)
NPAIR = SCN // 2        # 10 DoubleRow pairs (8 cache + 2 new)
VPW = 2 * 2 * 80        # per-pair bytes/partition in vp (eo x ko x 80)
SCALE2 = float(HD ** -0.5)  # fold both q and k scales into q

LAST_EXEC_NS = None
LAST_RESULTS = None
DEBUG = False


def _emit(ctx, tc, D):
    nc = tc.nc

    # ---------------- constants / persistent SBUF ----------------
    const = ctx.enter_context(tc.tile_pool(name="const", bufs=1))
    bqs_t = const.tile([128, 8], F32, name="bqs_t")
    bop_t = const.tile([128, 8], F32, name="bop_t")
    bvr_t = const.tile([1, NS], BF16, name="bvr_t")
    ones_bf = const.tile([1, 128], BF16, name="ones_bf")
    ones_fr = const.tile([65, 64], F32R, name="ones_fr")
    negc_t = const.tile([128, 1], F32, name="negc_t")
    nc.vector.memset(ones_bf[:], 1.0)
    nc.vector.memset(negc_t[:], -3.0)

    xT_t = const.tile([128, 4096], BF16, name="xT_t")  # k-chunk-major cols

    pers = ctx.enter_context(tc.tile_pool(name="pers", bufs=1))
    qT_t = pers.tile([128, 4096], BF16, name="qT_t")    # (x@Wq+bq)*s2, chunk-major
    knT_t = pers.tile([128, 4096], BF16, name="knT_t")  # (x@Wk)^T, chunk-major
    vnew_t = pers.tile([128, 4096], BF16, name="vnew_t")  # x@Wv+bv natural, tc-major
    outT_t = pers.tile([128, 4096], BF16, name="outT_t")

    wv_pool = ctx.enter_context(tc.tile_pool(name="wv", bufs=1))
    wv_tiles = [wv_pool.tile([128, 512], BF16, name=f"wv_{c}", tag=f"wv{c}")
                for c in range(NC)]
    p1_tiles = [wv_pool.tile([128, 512], BF16, name=f"p1_{m}", tag=f"p1{m}")
                for m in range(8)]

    wpool = ctx.enter_context(tc.tile_pool(name="wts", bufs=1))
    wq_t = [wpool.tile([128, 1024], BF16, name=f"wq{k}", tag=f"wq{k}")
            for k in range(8)]
    wk_t = [wpool.tile([128, 1024], BF16, name=f"wk{k}", tag=f"wk{k}")
            for k in range(8)]
    wvw_t = [wpool.tile([128, 1024], BF16, name=f"wvw{k}", tag=f"wvw{k}")
             for k in range(8)]
    wo_t = [wpool.tile([128, 1024], BF16, name=f"wo{k}", tag=f"wo{k}")
            for k in range(7)]
    wo7hi_t = wpool.tile([64, 1024], BF16, name="wo7hi", tag="wo7hi")
    wo7lo_t = wpool.tile([64, 1024], BF16, name="wo7lo", tag="wo7lo")

    kpool = ctx.enter_context(tc.tile_pool(name="kpair", bufs=2))
    vpool = ctx.enter_context(tc.tile_pool(name="vpair", bufs=2))
    probs = ctx.enter_context(tc.tile_pool(name="probs", bufs=10))
    stpool = ctx.enter_context(tc.tile_pool(name="stage", bufs=2))
    rpool = ctx.enter_context(tc.tile_pool(name="recip", bufs=2))
    tmpool = ctx.enter_context(tc.tile_pool(name="tmo", bufs=2))

    # one 3-deep rotation shared by scores, fillers, normalize and the tail:
    # 6 banks + 2 PV banks = all 8 PSUM banks; 3 bufs give the score matmuls
    # ~1.5 groups of lookahead so exp never waits on a PSUM bank
    spool = ctx.enter_context(tc.tile_pool(name="spsum", bufs=3, space="PSUM"))
    pvpool = ctx.enter_context(tc.tile_pool(name="pvpsum", bufs=1, space="PSUM"))

    def work_tile(name):
        return spool.tile([128, 1024], F32, name=name, tag="sc")[:, 0:512]

    # ---------------- DMA loads ----------------
    # ramp queue (gpsimd): xT first, then weights; q_burst(0)'s k-th matmul
    # chases the k-th Wq slice so the PE starts ~3.5us in
    nc.gpsimd.dma_start(ones_fr[:], D["onesf"].ap()[:, :])  # f32r cast dma
    nc.gpsimd.dma_start(xT_t[:, 0:2048], D["xT"].ap()[:, 0:2048])
    nc.gpsimd.dma_start(xT_t[:, 2048:4096], D["xT"].ap()[:, 2048:4096])
    # Wq slices split across both queues so issue serialization halves
    for k in range(8):
        q = nc.sync if k % 2 == 0 else nc.gpsimd
        q.dma_start(wq_t[k][:], D["Wq"].ap()[k * 128:(k + 1) * 128, :])
    for k in range(8):
        nc.gpsimd.dma_start(wk_t[k][:], D["Wk"].ap()[k * 128:(k + 1) * 128, :])
    for k in range(8):
        nc.gpsimd.dma_start(wvw_t[k][:], D["Wv"].ap()[k * 128:(k + 1) * 128, :])
    for k in range(7):
        nc.gpsimd.dma_start(wo_t[k][:], D["Wo"].ap()[k * 128:(k + 1) * 128, :])
    # chunk-7 Wo halves staged at partitions 0:64 so the tail can consume
    # the odd-half mul output (tm) directly without a partition-shift DMA
    nc.gpsimd.dma_start(wo7hi_t[:], D["Wo"].ap()[896:960, :])
    nc.gpsimd.dma_start(wo7lo_t[:], D["Wo"].ap()[960:1024, :])

    # second queue (sync): small consts + kv-cache chunk prefetches
    nc.sync.dma_start(bqs_t[:], D["bqs"][:, :])

    nc.sync.dma_start(bop_t[:], D["bop"][:, :])
    nc.sync.dma_start(bvr_t[:], D["bvr"][:, :])

    kp_tiles = [None] * NC
    vp_tiles = [None] * NC

    def prefetch(c):
        kp = kpool.tile([128, 2048], BF16, name="kp", tag="kp")
        nc.sync.dma_start(kp[:], D["kTc"].ap()[c * 128:(c + 1) * 128, :])
        vp = vpool.tile([128, 2600], BF16, name="vp", tag="vp")
        nc.sync.dma_start(vp[:, 0:2080],
                          D["vb"].ap()[c * 128:(c + 1) * 128, :])
        # ones slots of the new-token aug blocks
        nc.vector.memset(
            vp[:, 2080:2600].rearrange("p (tc h q) -> p tc h q", h=2, q=65)
            [:, :, :, 64:65], 1.0)
        kp_tiles[c], vp_tiles[c] = kp, vp

    prefetch(0)

    # ---------------- burst helpers (filler PE work) ----------------
    # each burst is split into two 4-matmul halves so a filler never blocks
    # the PE for more than ~0.9us between score groups
    def q_burst(m, half):
        ks = range(4) if half == 0 else range(4, 8)
        if half == 0:
            pt = work_tile(f"qp{m}")
            q_burst.pt[m] = pt
        pt = q_burst.pt[m]
        for k in ks:
            nc.tensor.matmul(pt[:], lhsT=wq_t[k][:, m * 128:(m + 1) * 128],
                             rhs=xT_t[:, k * 512:(k + 1) * 512],
                             start=(k == 0), stop=(k == 7))
        if half == 1:
            nc.vector.tensor_scalar(qT_t[:, m * 512:(m + 1) * 512], pt[:],
                                    SCALE2, bqs_t[:, m:m + 1], ALU.mult, ALU.add)
    q_burst.pt = {}

    def k_burst(c, half):
        ks = range(4) if half == 0 else range(4, 8)
        if half == 0:
            pt = work_tile(f"kp{c}")
            k_burst.pt[c] = pt
        pt = k_burst.pt[c]
        for k in ks:
            nc.tensor.matmul(pt[:], lhsT=wk_t[k][:, c * 128:(c + 1) * 128],
                             rhs=xT_t[:, k * 512:(k + 1) * 512],
                             start=(k == 0), stop=(k == 7))
        if half == 1:
            nc.vector.tensor_copy(knT_t[:, c * 512:(c + 1) * 512], pt[:])
            nc.sync.dma_start(D["keyT"].ap()[c * 128:(c + 1) * 128, :],
                              knT_t[:, c * 512:(c + 1) * 512])
    k_burst.pt = {}

    def v_burst4(cg, t4, half):
        # value[t4-block, chunks 4cg..4cg+3]
        ks = range(4) if half == 0 else range(4, 8)
        if half == 0:
            pt = work_tile(f"vb{cg}{t4}")
            v_burst4.pt[(cg, t4)] = pt
        pt = v_burst4.pt[(cg, t4)]
        for k in ks:
            nc.tensor.matmul(
                pt[:],
                lhsT=xT_t[:, k * 512 + t4 * 128:k * 512 + (t4 + 1) * 128],
                rhs=wvw_t[k][:, cg * 512:(cg + 1) * 512],
                start=(k == 0), stop=False)
        if half == 1:
            nc.tensor.matmul(pt[:], lhsT=ones_bf[0:1, 0:128],
                             rhs=bvr_t[0:1, cg * 512:(cg + 1) * 512],
                             start=False, stop=True)
            nc.vector.tensor_copy(
                vnew_t[:, t4 * 1024 + cg * 512:t4 * 1024 + (cg + 1) * 512],
                pt[:])
    v_burst4.pt = {}

    def vp_aug(c):
        vp = vp_tiles[c]
        for t4 in range(4):
            base = 2080 + t4 * 130
            so = t4 * 1024 + c * 128
            nc.vector.tensor_copy(vp[:, base:base + 64],
                                  vnew_t[:, so:so + 64])
            nc.vector.tensor_copy(vp[:, base + 65:base + 129],
                                  vnew_t[:, so + 64:so + 128])

    def op_a(m):
        # output-projection partial over chunks 0..3 (+ bias), chunk-5/6 filler
        pt = work_tile(f"mpa{m}")
        for cc in range(4):
            nc.tensor.matmul(pt[:], lhsT=wo_t[cc][:, m * 128:(m + 1) * 128],
                             rhs=wv_tiles[cc][:], start=(cc == 0),
                             stop=(cc == 3))
        nc.vector.tensor_scalar(p1_tiles[m][:], pt[:], 1.0,
                                bop_t[:, m:m + 1], ALU.mult, ALU.add)

    def op_b(m):
        # chunks 4..6 partial merged into p1 (chunk-7 filler)
        pt = work_tile(f"mpb{m}")
        for cc in range(4, 7):
            nc.tensor.matmul(pt[:], lhsT=wo_t[cc][:, m * 128:(m + 1) * 128],
                             rhs=wv_tiles[cc][:], start=(cc == 4),
                             stop=(cc == 6))
        nc.vector.tensor_add(p1_tiles[m][:], p1_tiles[m][:], pt[:])

    # normalize split: stage PSUM->SBUF + fast recips at end of chunk c;
    # broadcast matmuls + DVE muls early in chunk c+1 (or tail for c=7)
    chunk_state = {}
    tail_tm = [None]

    def stage_recips(c, pve, pvo):
        pse = stpool.tile([65, 512], F32, name=f"pse{c}", tag="pse")
        pso = stpool.tile([65, 512], F32, name=f"pso{c}", tag="pso")
        # denom rows first: the dma hop to partition 0 (recip_approx_fast
        # mislowers at base_partition 64) + recip overlap the big copies
        nc.vector.tensor_copy(pse[64:65, :], pve[64:65, :])
        nc.vector.tensor_copy(pso[64:65, :], pvo[64:65, :])
        den = rpool.tile([1, 1024], F32, name=f"den{c}", tag="den")
        nc.sync.dma_start(den[0:1, 0:512], pse[64:65, :])
        nc.sync.dma_start(den[0:1, 512:1024], pso[64:65, :])
        nc.vector.tensor_copy(pse[0:64, :], pve[0:64, :])
        nc.vector.tensor_copy(pso[0:64, :], pvo[0:64, :])
        rcp = rpool.tile([1, 1024], F32, name=f"rcp{c}", tag="rcp")
        rcf = rpool.tile([1, 1024], F32R, name=f"rcf{c}", tag="rcf")
        nc.vector.reciprocal_approx_fast(rcp[:], den[:])
        nc.vector.tensor_copy(rcf[:], rcp[:])
        if DEBUG and c == 0:
            nc.sync.dma_start(D["dbg_pse0"].ap()[:, :], pse[:])
            nc.sync.dma_start(D["dbg_pso0"].ap()[:, :], pso[:])
            nc.sync.dma_start(D["dbg_qT"].ap()[:, :], qT_t[:])
        chunk_state[c] = (pse, pso, rcf)

    def normalize(c):
        pse, pso, rcf = chunk_state.pop(c)
        sce = work_tile(f"sce{c}")
        nc.tensor.matmul(sce[0:64, :], lhsT=ones_fr[0:1, :],
                         rhs=rcf[0:1, 0:512], start=True, stop=True)
        sco = work_tile(f"sco{c}")
        nc.tensor.matmul(sco[0:64, :], lhsT=ones_fr[0:1, :],
                         rhs=rcf[0:1, 512:1024], start=True, stop=True)
        if DEBUG and c == 0:
            nc.sync.dma_start(D["dbg_rcp"].ap()[:, :], rcf[0:1, :].bitcast(F32))
            dbg_sce_t = stpool.tile([64, 512], F32, name="dbg_sce_t", tag="dbgs")
            nc.vector.tensor_copy(dbg_sce_t[:], sce[0:64, :])
            nc.sync.dma_start(D["dbg_sce"].ap()[:, :], dbg_sce_t[:])
        nc.vector.tensor_mul(wv_tiles[c][0:64, :], pse[0:64, :], sce[0:64, :])
        tm = tmpool.tile([64, 512], BF16, name=f"tm{c}", tag="tm")
        nc.vector.tensor_mul(tm[:], pso[0:64, :], sco[0:64, :])
        if c == NC - 1:
            tail_tm[0] = tm  # consumed directly by the tail matmuls
        else:
            nc.sync.dma_start(wv_tiles[c][64:128, :], tm[:])

    # ---------------- attention chunk pieces ----------------
    def scores_pair(c, g, kp):
        # 4 matmuls: (se,so) for j=2g then j=2g+1; se on PE rows 0:63 and so
        # on rows 64:127 emitted adjacently -> concurrent row-tiled execution
        se = spool.tile([128, 1024], F32, name="se", tag="sc")
        so = spool.tile([128, 1024], F32, name="so", tag="sc")
        rhs_e = qT_t[0:64, c * 512:(c + 1) * 512]
        rhs_o = qT_t[64:128, c * 512:(c + 1) * 512]
        lhs = []
        for jj in range(2):
            j = 2 * g + jj
            if j < 16:
                lhs.append((kp[0:64, j * 128:(j + 1) * 128],
                            kp[64:128, j * 128:(j + 1) * 128]))
            else:
                jo = c * 512 + (j - 16) * 128
                lhs.append((knT_t[0:64, jo:jo + 128],
                            knT_t[64:128, jo:jo + 128]))
        # both even-half matmuls FIRST: the odd tile's bank is freed one exp
        # later, and a waiting so-matmul in the strict-FIFO PE queue would
        # otherwise delay se jj1 (and with it the next even exp) every group
        for jj in range(2):
            nc.tensor.matmul(se[:, jj * 512:(jj + 1) * 512], lhsT=lhs[jj][0],
                             rhs=rhs_e, start=True, stop=True)
        for jj in range(2):
            nc.tensor.matmul(so[:, jj * 512:(jj + 1) * 512], lhsT=lhs[jj][1],
                             rhs=rhs_o, start=True, stop=True)
        pe_t = probs.tile([128, 1024], BF16, name="pe", tag="pr")
        nc.scalar.activation(pe_t[:], se[:], ACTF.Exp)
        po_t = probs.tile([128, 1024], BF16, name="po", tag="pr")
        nc.scalar.activation(po_t[:], so[:], ACTF.Exp)
        return pe_t, po_t

    def pv_pair(g, pr, vp, pve, pvo):
        pe_t, po_t = pr
        for jj in range(2):
            j = 2 * g + jj
            nc.tensor.matmul(pve[:], lhsT=vp[:, j * 130:j * 130 + 65],
                             rhs=pe_t[:, jj * 512:(jj + 1) * 512],
                             start=(j == 0), stop=(j == SCN - 1))
            nc.tensor.matmul(pvo[:], lhsT=vp[:, j * 130 + 65:(j + 1) * 130],
                             rhs=po_t[:, jj * 512:(jj + 1) * 512],
                             start=(j == 0), stop=(j == SCN - 1))

    # ---------------- head: q chunk 0 ----------------
    q_burst(0, 0)
    q_burst(0, 1)

    # ---------------- main chunk loop ----------------
    # filler half-burst items per chunk, consumed one per group slot
    def halves(*items):
        out = []
        for it in items:
            if isinstance(it, tuple):
                f, args = it
                out.append(lambda f=f, a=args: f(*a, 0))
                out.append(lambda f=f, a=args: f(*a, 1))
            else:
                out.append(it)
        return out

    chunk_fillers = {
        0: halves((q_burst, (1,)), (k_burst, (0,)), (v_burst4, (0, 0)),
                  (v_burst4, (0, 1)), (v_burst4, (0, 2)), (v_burst4, (0, 3)),
                  lambda: vp_aug(0)),
        1: halves((q_burst, (2,)), (k_burst, (1,)), (v_burst4, (1, 0)),
                  lambda: vp_aug(1)),
        2: halves((q_burst, (3,)), (k_burst, (2,)), (v_burst4, (1, 1)),
                  (v_burst4, (1, 2)), lambda: vp_aug(2)),
        3: halves((q_burst, (4,)), (k_burst, (3,)), (v_burst4, (1, 3)),
                  lambda: vp_aug(3)),
        4: halves((q_burst, (5,)), (k_burst, (4,)), lambda: vp_aug(4)),
        5: halves((q_burst, (6,)), (k_burst, (5,)), lambda: vp_aug(5)) + [
            (lambda m=m: op_a(m)) for m in range(4)],
        6: halves((q_burst, (7,)), (k_burst, (6,)), lambda: vp_aug(6)) + [
            (lambda m=m: op_a(m)) for m in range(4, 8)],
        7: halves((k_burst, (7,)), lambda: vp_aug(7)) + [
            (lambda m=m: op_b(m)) for m in range(8)],
    }

    for c in range(NC):
        kp, vp = kp_tiles[c], vp_tiles[c]
        pr = [None] * NPAIR
        fills = iter(chunk_fillers[c])

        def fill(n=1):
            for _ in range(n):
                f = next(fills, None)
                if f is not None:
                    f()

        # flat software pipeline with a 4-group PV lag: a PV matmul only
        # enters the strict-FIFO PE queue when its exp is long done, so the
        # queue never parks; the previous chunk's pv(6..9) ride under this
        # chunk's first four score groups
        for g in range(NPAIR):
            pr[g] = scores_pair(c, g, kp)
            if DEBUG and c == 0 and g == 0:
                nc.sync.dma_start(D["dbg_pe0"].ap()[:, :], pr[0][0][:])
                nc.sync.dma_start(D["dbg_po0"].ap()[:, :], pr[0][1][:])
            if c > 0 and g < 4:
                pv_pair(6 + g, prev_pr[6 + g], prev_vp, prev_pve, prev_pvo)
                if g == 3:
                    stage_recips(c - 1, prev_pve, prev_pvo)
                    if c + 1 < NC:
                        prefetch(c + 1)
            if g == 4:
                pve = pvpool.tile([65, 512], F32, name="pve", tag="pv_e")
                pvo = pvpool.tile([65, 512], F32, name="pvo", tag="pv_o")
            if g >= 4:
                pv_pair(g - 4, pr[g - 4], vp, pve, pvo)
            # normalize(c-1) after rcf(c-1) (emitted g==3) has ~2 groups of
            # slack so its sce matmul never blocks the PE queue; chunk 7's
            # op_b fillers (first consumed at g==5) need wv up to 6 first
            if g == 5 and c > 0:
                normalize(c - 1)
            if g == 5 and c == 0:
                prefetch(1)
            if c == 0:
                fill(2)
            elif c == NC - 1:
                fill((1, 1, 1, 0, 0, 2, 2, 2, 2, 2)[g])
            else:
                fill(1)
        fill(20)  # drain any leftovers
        if c == 3:
            nc.sync.dma_start(
                D["value"].ap().rearrange("(tc p) o -> p tc o", p=128),
                vnew_t[:].rearrange("p (tc o) -> p tc o", tc=4))
        prev_pr, prev_vp, prev_pve, prev_pvo = pr, vp, pve, pvo

    # ---------------- tail ----------------
    for g in range(4):
        pv_pair(6 + g, prev_pr[6 + g], prev_vp, prev_pve, prev_pvo)
    stage_recips(NC - 1, prev_pve, prev_pvo)
    normalize(NC - 1)
    if DEBUG:
        nc.sync.dma_start(D["dbg_wv0"].ap()[:, :], wv_tiles[0][:])
    tm7 = tail_tm[0]
    for m in range(8):
        pt = work_tile(f"op{m}")
        nc.tensor.matmul(pt[:], lhsT=wo7hi_t[:, m * 128:(m + 1) * 128],
                         rhs=wv_tiles[7][0:64, :], start=True, stop=False)
        nc.tensor.matmul(pt[:], lhsT=wo7lo_t[:, m * 128:(m + 1) * 128],
                         rhs=tm7[:], start=False, stop=True)
        # outT = pt + p1   (p1 already carries bias + chunks 0..6)
        nc.vector.tensor_add(outT_t[:, m * 512:(m + 1) * 512], pt[:],
                             p1_tiles[m][:])
        nc.sync.dma_start(
            D["outT"].ap()[m * 128:(m + 1) * 128, :],
            outT_t[:, m * 512:(m + 1) * 512])


def build():
    nc = bacc.Bacc("TRN2", target_bir_lowering=False, debug=False)
    D = {}
    D["xT"] = nc.dram_tensor("xT", [128, 4096], BF16, kind="ExternalInput")
    D["kTc"] = nc.dram_tensor("kTc", [NS, S], BF16, kind="ExternalInput")
    D["vb"] = nc.dram_tensor("vb", [NC * 128, 2080], BF16,
                             kind="ExternalInput")
    for w in ("Wq", "Wk", "Wv", "Wo"):
        D[w] = nc.dram_tensor(w, [NS, NS], BF16, kind="ExternalInput")
    D["bqs"] = nc.dram_tensor("bqs", [128, 8], F32, kind="ExternalInput")
    D["bop"] = nc.dram_tensor("bop", [128, 8], F32, kind="ExternalInput")
    D["bvr"] = nc.dram_tensor("bvr", [1, NS], BF16, kind="ExternalInput")
    D["onesf"] = nc.dram_tensor("onesf", [65, 64], F32, kind="ExternalInput")
    if DEBUG:
        D["dbg_qT"] = nc.dram_tensor("dbg_qT", [128, 4096], BF16, kind="ExternalOutput")
        D["dbg_pe0"] = nc.dram_tensor("dbg_pe0", [128, 1024], BF16, kind="ExternalOutput")
        D["dbg_po0"] = nc.dram_tensor("dbg_po0", [128, 1024], BF16, kind="ExternalOutput")
        D["dbg_pse0"] = nc.dram_tensor("dbg_pse0", [65, 512], F32, kind="ExternalOutput")
        D["dbg_pso0"] = nc.dram_tensor("dbg_pso0", [65, 512], F32, kind="ExternalOutput")
        D["dbg_wv0"] = nc.dram_tensor("dbg_wv0", [128, 512], BF16, kind="ExternalOutput")
        D["dbg_rcp"] = nc.dram_tensor("dbg_rcp", [1, 1024], F32, kind="ExternalOutput")
        D["dbg_sce"] = nc.dram_tensor("dbg_sce", [64, 512], F32, kind="ExternalOutput")
    D["outT"] = nc.dram_tensor("outT", [NS, T], BF16, kind="ExternalOutput")
    D["keyT"] = nc.dram_tensor("keyT", [NS, T], BF16, kind="ExternalOutput")
    D["value"] = nc.dram_tensor("value", [T, NS], BF16, kind="ExternalOutput")

    with tile.TileContext(nc) as tc:
        with ExitStack() as ctx:
            _emit(ctx, tc, D)
    nc.compile()
    return nc


_NC_CACHE = None


def _get_nc():
    global _NC_CACHE
    if _NC_CACHE is None:
        _NC_CACHE = build()
    return _NC_CACHE


def prep_core_inputs(b, x, kv_cache, WqB, WkB, WvB, WoB, bqs, bop, bvr):
    xT = np.ascontiguousarray(x[b].T).reshape(8, 128, 512) \
        .transpose(1, 0, 2).reshape(128, 4096).astype(NPBF)
    kTc = np.ascontiguousarray(kv_cache[b, 0, 0].T).astype(NPBF)  # [NS, S]
    vjp = kv_cache[b, 0, 1].reshape(16, 128, NH, HD)  # [j, p, h, d]
    vh = vjp.transpose(2, 1, 0, 3)                    # [h, p, j, d]
    vb = np.ones((NC, 128, 16, 130), NPBF)
    vb[..., 0:64] = vh[0::2]
    vb[..., 65:129] = vh[1::2]
    return {
        "xT": xT, "kTc": kTc, "vb": vb.reshape(NC * 128, 2080),
        "Wq": WqB, "Wk": WkB, "Wv": WvB, "Wo": WoB,
        "bqs": bqs, "bop": bop, "bvr": bvr,
        "onesf": np.ones((65, 64), np.float32),
    }


def kernel(x, kv_cache, offset=0, Wq=None, bq=None, Wk=None, Wv=None, bv=None,
           Wo=None, bo=None, trace=False):
    global LAST_EXEC_NS, LAST_RESULTS
    x = np.asarray(x, np.float32)
    kv_cache = np.asarray(kv_cache, np.float32)
    Wq, bq, Wk, Wv, bv, Wo, bo = [np.asarray(a, np.float32)
                                  for a in (Wq, bq, Wk, Wv, bv, Wo, bo)]
    WqB, WkB, WvB, WoB = [w.astype(NPBF) for w in (Wq, Wk, Wv, Wo)]
    bqs = np.ascontiguousarray((bq * SCALE2).reshape(8, 128).T)
    bop = np.ascontiguousarray(bo.reshape(8, 128).T)
    bvr = bv[None, :].astype(NPBF)
    in_maps = [prep_core_inputs(b, x, kv_cache, WqB, WkB, WvB, WoB,
                                bqs, bop, bvr) for b in range(B)]
    nc = _get_nc()
    res = run_bass_kernel_spmd(nc, in_maps, core_ids=list(range(B)), trace=trace)
    LAST_EXEC_NS = res.exec_time_ns
    LAST_RESULTS = res
    out = np.stack([res.results[b]["outT"].astype(np.float32).T
                    for b in range(B)])
    key = np.stack([res.results[b]["keyT"].astype(np.float32).T
                    for b in range(B)])
    value = np.stack([res.results[b]["value"].astype(np.float32)
                      for b in range(B)])
    return (np.ascontiguousarray(out), np.ascontiguousarray(key),
            np.ascontiguousarray(value))


# revision 25
# speedup vs baseline: 1.0491x; 1.0231x over previous
"""Trainium2 Bass kernel: cached multi-head self-attention decoder block.

Per-core (batch-parallel, B=8 -> 8 cores), fully fused single loop.
v2 design (ACT-exp-bound ~178us target):
  - scores: even/odd head matmuls placed on PE row-halves (tile rows 0:64 /
    64:128) and emitted adjacently -> hardware runs them CONCURRENTLY
  - probs exp'd by ACT directly to fp8e4; PV runs fp8 DoubleRow (2 s-blocks
    per matmul, 2 fp8 MACs/cell/cycle) with the v-cache host-interleaved as
    [pair, eo, ko, 80] fp8 (col 64 = ones -> softmax denominator row 64)
  - per chunk, PV PSUM is immediately staged to SBUF (frees banks for the
    next chunk -> no PE stall, no HAM re-throttle); denominators inverted
    with reciprocal_approx_fast; normalize via f32r ones-broadcast matmul
  - DMA: xT/Wq k-slices interleaved on a dedicated queue so the first
    projection matmul starts ~1us in and chases the DMA
  - output projection: partials over chunks 0..6 computed as chunk-7 PE
    filler, tail only does the last chunk's matmul + fused
    (psum+bias)+partial merge, outT DMA'd per m-slice
"""

import numpy as np
import ml_dtypes
from contextlib import ExitStack

import concourse.bass as bass
import concourse.tile as tile
from concourse import bacc, mybir
from concourse.bass_utils import run_bass_kernel_spmd

F32 = mybir.dt.float32
F32R = mybir.dt.float32r
BF16 = mybir.dt.bfloat16
FP8 = mybir.dt.float8e4
ALU = mybir.AluOpType
ACTF = mybir.ActivationFunctionType
DR = mybir.MatmulPerfMode.DoubleRow
NPBF = ml_dtypes.bfloat16
NPF8 = ml_dtypes.float8_e4m3

B, T, S, NS, NH, HD = 8, 512, 2048, 1024, 16, 64
ST = S + T              # 2560
NC = NS // 128          # 8 chunks (head pairs)
SCN = ST // 128         # 20 s-blocks (16 cache + 4 new)
NPAIR = SCN // 2        # 10 DoubleRow pairs (8 cache + 2 new)
VPW = 2 * 2 * 80        # per-pair bytes/partition in vp (eo x ko x 80)
SCALE2 = float(HD ** -0.5)  # fold both q and k scales into q

LAST_EXEC_NS = None
LAST_RESULTS = None
DEBUG = False


def _emit(ctx, tc, D):
    nc = tc.nc

    # ---------------- constants / persistent SBUF ----------------
    const = ctx.enter_context(tc.tile_pool(name="const", bufs=1))
    bqs_t = const.tile([128, 8], F32, name="bqs_t")
    bop_t = const.tile([128, 8], F32, name="bop_t")
    bvr_t = const.tile([1, NS], BF16, name="bvr_t")
    ones_bf = const.tile([1, 128], BF16, name="ones_bf")
    nc.vector.memset(ones_bf[:], 1.0)

    xT_t = const.tile([128, 4096], BF16, name="xT_t")  # k-chunk-major cols

    pers = ctx.enter_context(tc.tile_pool(name="pers", bufs=1))
    qT_t = pers.tile([128, 4096], BF16, name="qT_t")    # (x@Wq+bq)*s2, chunk-major
    knT_t = pers.tile([128, 4096], BF16, name="knT_t")  # (x@Wk)^T, chunk-major
    vnew_t = pers.tile([128, 4096], BF16, name="vnew_t")  # x@Wv+bv natural, tc-major
    outT_t = pers.tile([128, 4096], BF16, name="outT_t")

    wv_pool = ctx.enter_context(tc.tile_pool(name="wv", bufs=1))
    wv_tiles = [wv_pool.tile([128, 512], BF16, name=f"wv_{c}", tag=f"wv{c}")
                for c in range(NC)]
    p1_tiles = [wv_pool.tile([128, 512], BF16, name=f"p1_{m}", tag=f"p1{m}")
                for m in range(8)]

    wpool = ctx.enter_context(tc.tile_pool(name="wts", bufs=1))
    wq_t = [wpool.tile([128, 1024], BF16, name=f"wq{k}", tag=f"wq{k}")
            for k in range(8)]
    wk_t = [wpool.tile([128, 1024], BF16, name=f"wk{k}", tag=f"wk{k}")
            for k in range(8)]
    wvw_t = [wpool.tile([128, 1024], BF16, name=f"wvw{k}", tag=f"wvw{k}")
             for k in range(8)]
    wo_t = [wpool.tile([128, 1024], BF16, name=f"wo{k}", tag=f"wo{k}")
            for k in range(7)]
    wo7hi_t = wpool.tile([64, 1024], BF16, name="wo7hi", tag="wo7hi")
    wo7lo_t = wpool.tile([64, 1024], BF16, name="wo7lo", tag="wo7lo")

    kpool = ctx.enter_context(tc.tile_pool(name="kpair", bufs=2))
    vpool = ctx.enter_context(tc.tile_pool(name="vpair", bufs=2))
    probs = ctx.enter_context(tc.tile_pool(name="probs", bufs=10))
    stpool = ctx.enter_context(tc.tile_pool(name="stage", bufs=2))
    rpool = ctx.enter_context(tc.tile_pool(name="recip", bufs=2))
    tmpool = ctx.enter_context(tc.tile_pool(name="tmo", bufs=2))

    # one 3-deep rotation shared by scores, fillers, normalize and the tail:
    # 6 banks + 2 PV banks = all 8 PSUM banks; 3 bufs give the score matmuls
    # ~1.5 groups of lookahead so exp never waits on a PSUM bank
    spool = ctx.enter_context(tc.tile_pool(name="spsum", bufs=3, space="PSUM"))
    pvpool = ctx.enter_context(tc.tile_pool(name="pvpsum", bufs=1, space="PSUM"))

    def work_tile(name):
        return spool.tile([128, 1024], F32, name=name, tag="sc")[:, 0:512]

    # ---------------- DMA loads ----------------
    # ramp queue (gpsimd): xT first, then weights; q_burst(0)'s k-th matmul
    # chases the k-th Wq slice so the PE starts ~3.5us in
    nc.gpsimd.dma_start(xT_t[:, 0:2048], D["xT"].ap()[:, 0:2048])
    nc.gpsimd.dma_start(xT_t[:, 2048:4096], D["xT"].ap()[:, 2048:4096])
    # Wq slices split across both queues so issue serialization halves
    for k in range(8):
        q = nc.sync if k % 2 == 0 else nc.gpsimd
        q.dma_start(wq_t[k][:], D["Wq"].ap()[k * 128:(k + 1) * 128, :])
    for k in range(8):
        nc.gpsimd.dma_start(wk_t[k][:], D["Wk"].ap()[k * 128:(k + 1) * 128, :])
    for k in range(8):
        nc.gpsimd.dma_start(wvw_t[k][:], D["Wv"].ap()[k * 128:(k + 1) * 128, :])
    for k in range(7):
        nc.gpsimd.dma_start(wo_t[k][:], D["Wo"].ap()[k * 128:(k + 1) * 128, :])
    # chunk-7 Wo halves staged at partitions 0:64 so the tail can consume
    # the odd-half mul output (tm) directly without a partition-shift DMA
    nc.gpsimd.dma_start(wo7hi_t[:], D["Wo"].ap()[896:960, :])
    nc.gpsimd.dma_start(wo7lo_t[:], D["Wo"].ap()[960:1024, :])

    # second queue (sync): small consts + kv-cache chunk prefetches
    nc.sync.dma_start(bqs_t[:], D["bqs"][:, :])

    nc.sync.dma_start(bop_t[:], D["bop"][:, :])
    nc.sync.dma_start(bvr_t[:], D["bvr"][:, :])

    kp_tiles = [None] * NC
    vp_tiles = [None] * NC

    def prefetch(c):
        kp = kpool.tile([128, 2048], BF16, name="kp", tag="kp")
        nc.sync.dma_start(kp[:], D["kTc"].ap()[c * 128:(c + 1) * 128, :])
        vp = vpool.tile([128, 2600], BF16, name="vp", tag="vp")
        nc.sync.dma_start(vp[:, 0:2080],
                          D["vb"].ap()[c * 128:(c + 1) * 128, :])
        # ones slots of the new-token aug blocks
        nc.vector.memset(
            vp[:, 2080:2600].rearrange("p (tc h q) -> p tc h q", h=2, q=65)
            [:, :, :, 64:65], 1.0)
        kp_tiles[c], vp_tiles[c] = kp, vp

    prefetch(0)

    # ---------------- burst helpers (filler PE work) ----------------
    # each burst is split into two 4-matmul halves so a filler never blocks
    # the PE for more than ~0.9us between score groups
    def q_burst(m, half):
        ks = range(4) if half == 0 else range(4, 8)
        if half == 0:
            pt = work_tile(f"qp{m}")
            q_burst.pt[m] = pt
        pt = q_burst.pt[m]
        for k in ks:
            nc.tensor.matmul(pt[:], lhsT=wq_t[k][:, m * 128:(m + 1) * 128],
                             rhs=xT_t[:, k * 512:(k + 1) * 512],
                             start=(k == 0), stop=(k == 7))
        if half == 1:
            nc.vector.tensor_scalar(qT_t[:, m * 512:(m + 1) * 512], pt[:],
                                    SCALE2, bqs_t[:, m:m + 1], ALU.mult, ALU.add)
    q_burst.pt = {}

    def k_burst(c, half):
        ks = range(4) if half == 0 else range(4, 8)
        if half == 0:
            pt = work_tile(f"kp{c}")
            k_burst.pt[c] = pt
        pt = k_burst.pt[c]
        for k in ks:
            nc.tensor.matmul(pt[:], lhsT=wk_t[k][:, c * 128:(c + 1) * 128],
                             rhs=xT_t[:, k * 512:(k + 1) * 512],
                             start=(k == 0), stop=(k == 7))
        if half == 1:
            nc.vector.tensor_copy(knT_t[:, c * 512:(c + 1) * 512], pt[:])
            nc.sync.dma_start(D["keyT"].ap()[c * 128:(c + 1) * 128, :],
                              knT_t[:, c * 512:(c + 1) * 512])
    k_burst.pt = {}

    def v_burst4(cg, t4, half):
        # value[t4-block, chunks 4cg..4cg+3]
        ks = range(4) if half == 0 else range(4, 8)
        if half == 0:
            pt = work_tile(f"vb{cg}{t4}")
            v_burst4.pt[(cg, t4)] = pt
        pt = v_burst4.pt[(cg, t4)]
        for k in ks:
            nc.tensor.matmul(
                pt[:],
                lhsT=xT_t[:, k * 512 + t4 * 128:k * 512 + (t4 + 1) * 128],
                rhs=wvw_t[k][:, cg * 512:(cg + 1) * 512],
                start=(k == 0), stop=False)
        if half == 1:
            nc.tensor.matmul(pt[:], lhsT=ones_bf[0:1, 0:128],
                             rhs=bvr_t[0:1, cg * 512:(cg + 1) * 512],
                             start=False, stop=True)
            nc.vector.tensor_copy(
                vnew_t[:, t4 * 1024 + cg * 512:t4 * 1024 + (cg + 1) * 512],
                pt[:])
    v_burst4.pt = {}

    def vp_aug(c):
        vp = vp_tiles[c]
        for t4 in range(4):
            base = 2080 + t4 * 130
            so = t4 * 1024 + c * 128
            nc.vector.tensor_copy(vp[:, base:base + 64],
                                  vnew_t[:, so:so + 64])
            nc.vector.tensor_copy(vp[:, base + 65:base + 129],
                                  vnew_t[:, so + 64:so + 128])

    def op_a(m):
        # output-projection partial over chunks 0..3 (+ bias), chunk-5/6 filler
        pt = work_tile(f"mpa{m}")
        for cc in range(4):
            nc.tensor.matmul(pt[:], lhsT=wo_t[cc][:, m * 128:(m + 1) * 128],
                             rhs=wv_tiles[cc][:], start=(cc == 0),
                             stop=(cc == 3))
        nc.vector.tensor_scalar(p1_tiles[m][:], pt[:], 1.0,
                                bop_t[:, m:m + 1], ALU.mult, ALU.add)

    def op_b(m):
        # chunks 4..6 partial merged into p1 (chunk-7 filler)
        pt = work_tile(f"mpb{m}")
        for cc in range(4, 7):
            nc.tensor.matmul(pt[:], lhsT=wo_t[cc][:, m * 128:(m + 1) * 128],
                             rhs=wv_tiles[cc][:], start=(cc == 4),
                             stop=(cc == 6))
        nc.vector.tensor_add(p1_tiles[m][:], p1_tiles[m][:], pt[:])

    # normalize split: stage PSUM->SBUF + fast recips at end of chunk c;
    # broadcast matmuls + DVE muls early in chunk c+1 (or tail for c=7)
    chunk_state = {}
    tail_tm = [None]

    def stage_recips(c, pve, pvo):
        pse = stpool.tile([65, 512], F32, name=f"pse{c}", tag="pse")
        pso = stpool.tile([65, 512], F32, name=f"pso{c}", tag="pso")
        # denom rows first: the dma hop to partition 0 (recip_approx_fast
        # mislowers at base_partition 64) + recip overlap the big copies
        nc.vector.tensor_copy(pse[64:65, :], pve[64:65, :])
        nc.vector.tensor_copy(pso[64:65, :], pvo[64:65, :])
        den = rpool.tile([1, 1024], F32, name=f"den{c}", tag="den")
        nc.gpsimd.dma_start(den[0:1, 0:512], pse[64:65, :])
        nc.gpsimd.dma_start(den[0:1, 512:1024], pso[64:65, :])
        nc.vector.tensor_copy(pse[0:64, :], pve[0:64, :])
        nc.vector.tensor_copy(pso[0:64, :], pvo[0:64, :])
        rcp = rpool.tile([1, 1024], F32, name=f"rcp{c}", tag="rcp")
        nc.vector.reciprocal_approx_fast(rcp[:], den[:])
        # broadcast 1/denom across 64 partitions on the (idle) gpsimd engine
        # -> the whole normalize chain stays off the PE queue
        scl = rpool.tile([64, 1024], F32, name=f"scl{c}", tag="scl")
        nc.gpsimd.partition_broadcast(scl[:], rcp[0:1, :], channels=64)
        if DEBUG and c == 0:
            nc.sync.dma_start(D["dbg_pse0"].ap()[:, :], pse[:])
            nc.sync.dma_start(D["dbg_pso0"].ap()[:, :], pso[:])
            nc.sync.dma_start(D["dbg_qT"].ap()[:, :], qT_t[:])
        chunk_state[c] = (pse, pso, scl)

    def normalize(c):
        pse, pso, scl = chunk_state.pop(c)
        if DEBUG and c == 0:
            nc.sync.dma_start(D["dbg_rcp"].ap()[:, :], scl[0:1, :])
            nc.sync.dma_start(D["dbg_sce"].ap()[:, :], scl[:, 0:512])
        nc.vector.tensor_mul(wv_tiles[c][0:64, :], pse[0:64, :],
                             scl[:, 0:512])
        tm = tmpool.tile([64, 512], BF16, name=f"tm{c}", tag="tm")
        nc.vector.tensor_mul(tm[:], pso[0:64, :], scl[:, 512:1024])
        if c == NC - 1:
            tail_tm[0] = tm  # consumed directly by the tail matmuls
        else:
            nc.sync.dma_start(wv_tiles[c][64:128, :], tm[:])

    # ---------------- attention chunk pieces ----------------
    def scores_pair(c, g, kp):
        # 4 matmuls: (se,so) for j=2g then j=2g+1; se on PE rows 0:63 and so
        # on rows 64:127 emitted adjacently -> concurrent row-tiled execution
        se = spool.tile([128, 1024], F32, name="se", tag="sc")
        so = spool.tile([128, 1024], F32, name="so", tag="sc")
        rhs_e = qT_t[0:64, c * 512:(c + 1) * 512]
        rhs_o = qT_t[64:128, c * 512:(c + 1) * 512]
        lhs = []
        for jj in range(2):
            j = 2 * g + jj
            if j < 16:
                lhs.append((kp[0:64, j * 128:(j + 1) * 128],
                            kp[64:128, j * 128:(j + 1) * 128]))
            else:
                jo = c * 512 + (j - 16) * 128
                lhs.append((knT_t[0:64, jo:jo + 128],
                            knT_t[64:128, jo:jo + 128]))
        # both even-half matmuls FIRST: the odd tile's bank is freed one exp
        # later, and a waiting so-matmul in the strict-FIFO PE queue would
        # otherwise delay se jj1 (and with it the next even exp) every group
        for jj in range(2):
            nc.tensor.matmul(se[:, jj * 512:(jj + 1) * 512], lhsT=lhs[jj][0],
                             rhs=rhs_e, start=True, stop=True)
        for jj in range(2):
            nc.tensor.matmul(so[:, jj * 512:(jj + 1) * 512], lhsT=lhs[jj][1],
                             rhs=rhs_o, start=True, stop=True)
        pe_t = probs.tile([128, 1024], BF16, name="pe", tag="pr")
        nc.scalar.activation(pe_t[:], se[:], ACTF.Exp)
        po_t = probs.tile([128, 1024], BF16, name="po", tag="pr")
        nc.scalar.activation(po_t[:], so[:], ACTF.Exp)
        return pe_t, po_t

    def pv_pair(g, pr, vp, pve, pvo):
        pe_t, po_t = pr
        for jj in range(2):
            j = 2 * g + jj
            nc.tensor.matmul(pve[:], lhsT=vp[:, j * 130:j * 130 + 65],
                             rhs=pe_t[:, jj * 512:(jj + 1) * 512],
                             start=(j == 0), stop=(j == SCN - 1))
            nc.tensor.matmul(pvo[:], lhsT=vp[:, j * 130 + 65:(j + 1) * 130],
                             rhs=po_t[:, jj * 512:(jj + 1) * 512],
                             start=(j == 0), stop=(j == SCN - 1))

    # ---------------- head: q chunk 0 ----------------
    q_burst(0, 0)
    q_burst(0, 1)

    # ---------------- main chunk loop ----------------
    # filler half-burst items per chunk, consumed one per group slot
    def halves(*items):
        out = []
        for it in items:
            if isinstance(it, tuple):
                f, args = it
                out.append(lambda f=f, a=args: f(*a, 0))
                out.append(lambda f=f, a=args: f(*a, 1))
            else:
                out.append(it)
        return out

    chunk_fillers = {
        0: halves((q_burst, (1,)), (k_burst, (0,)), (v_burst4, (0, 0)),
                  (v_burst4, (0, 1)), (v_burst4, (0, 2)), (v_burst4, (0, 3)),
                  lambda: vp_aug(0)),
        1: halves((q_burst, (2,)), (k_burst, (1,)), (v_burst4, (1, 0)),
                  lambda: vp_aug(1)),
        2: halves((q_burst, (3,)), (k_burst, (2,)), (v_burst4, (1, 1)),
                  (v_burst4, (1, 2)), lambda: vp_aug(2)),
        3: halves((q_burst, (4,)), (k_burst, (3,)), (v_burst4, (1, 3)),
                  lambda: vp_aug(3)),
        4: halves((q_burst, (5,)), (k_burst, (4,)), lambda: vp_aug(4)),
        5: halves((q_burst, (6,)), (k_burst, (5,)), lambda: vp_aug(5)) + [
            (lambda m=m: op_a(m)) for m in range(4)],
        6: halves((q_burst, (7,)), (k_burst, (6,)), lambda: vp_aug(6)) + [
            (lambda m=m: op_a(m)) for m in range(4, 8)],
        7: halves((k_burst, (7,)), lambda: vp_aug(7)) + [
            (lambda m=m: op_b(m)) for m in range(8)],
    }

    for c in range(NC):
        kp, vp = kp_tiles[c], vp_tiles[c]
        pr = [None] * NPAIR
        fills = iter(chunk_fillers[c])

        def fill(n=1):
            for _ in range(n):
                f = next(fills, None)
                if f is not None:
                    f()

        # flat software pipeline with a 4-group PV lag: a PV matmul only
        # enters the strict-FIFO PE queue when its exp is long done, so the
        # queue never parks; the previous chunk's pv(6..9) ride under this
        # chunk's first four score groups
        for g in range(NPAIR):
            pr[g] = scores_pair(c, g, kp)
            if DEBUG and c == 0 and g == 0:
                nc.sync.dma_start(D["dbg_pe0"].ap()[:, :], pr[0][0][:])
                nc.sync.dma_start(D["dbg_po0"].ap()[:, :], pr[0][1][:])
            if c > 0 and g < 4:
                pv_pair(6 + g, prev_pr[6 + g], prev_vp, prev_pve, prev_pvo)
                if g == 3:
                    stage_recips(c - 1, prev_pve, prev_pvo)
                    if c + 1 < NC:
                        prefetch(c + 1)
            if g == 4:
                pve = pvpool.tile([65, 512], F32, name="pve", tag="pv_e")
                pvo = pvpool.tile([65, 512], F32, name="pvo", tag="pv_o")
            if g >= 4:
                pv_pair(g - 4, pr[g - 4], vp, pve, pvo)
            # normalize(c-1) after rcf(c-1) (emitted g==3) has ~2 groups of
            # slack so its sce matmul never blocks the PE queue; chunk 7's
            # op_b fillers (first consumed at g==5) need wv up to 6 first
            if g == 5 and c > 0:
                normalize(c - 1)
            if g == 5 and c == 0:
                prefetch(1)
            if c == 0:
                fill(2)
            elif c == NC - 1:
                fill((1, 1, 1, 0, 0, 2, 2, 2, 2, 2)[g])
            else:
                fill(1)
        fill(20)  # drain any leftovers
        if c == 3:
            nc.sync.dma_start(
                D["value"].ap().rearrange("(tc p) o -> p tc o", p=128),
                vnew_t[:].rearrange("p (tc o) -> p tc o", tc=4))
        prev_pr, prev_vp, prev_pve, prev_pvo = pr, vp, pve, pvo

    # ---------------- tail ----------------
    for g in range(4):
        pv_pair(6 + g, prev_pr[6 + g], prev_vp, prev_pve, prev_pvo)
    stage_recips(NC - 1, prev_pve, prev_pvo)
    normalize(NC - 1)
    if DEBUG:
        nc.sync.dma_start(D["dbg_wv0"].ap()[:, :], wv_tiles[0][:])
    tm7 = tail_tm[0]
    for m in range(8):
        pt = work_tile(f"op{m}")
        nc.tensor.matmul(pt[:], lhsT=wo7hi_t[:, m * 128:(m + 1) * 128],
                         rhs=wv_tiles[7][0:64, :], start=True, stop=False)
        nc.tensor.matmul(pt[:], lhsT=wo7lo_t[:, m * 128:(m + 1) * 128],
                         rhs=tm7[:], start=False, stop=True)
        # outT = pt + p1   (p1 already carries bias + chunks 0..6)
        nc.vector.tensor_add(outT_t[:, m * 512:(m + 1) * 512], pt[:],
                             p1_tiles[m][:])
        nc.sync.dma_start(
            D["outT"].ap()[m * 128:(m + 1) * 128, :],
            outT_t[:, m * 512:(m + 1) * 512])


def build():
    nc = bacc.Bacc("TRN2", target_bir_lowering=False, debug=False)
    D = {}
    D["xT"] = nc.dram_tensor("xT", [128, 4096], BF16, kind="ExternalInput")
    D["kTc"] = nc.dram_tensor("kTc", [NS, S], BF16, kind="ExternalInput")
    D["vb"] = nc.dram_tensor("vb", [NC * 128, 2080], BF16,
                             kind="ExternalInput")
    for w in ("Wq", "Wk", "Wv", "Wo"):
        D[w] = nc.dram_tensor(w, [NS, NS], BF16, kind="ExternalInput")
    D["bqs"] = nc.dram_tensor("bqs", [128, 8], F32, kind="ExternalInput")
    D["bop"] = nc.dram_tensor("bop", [128, 8], F32, kind="ExternalInput")
    D["bvr"] = nc.dram_tensor("bvr", [1, NS], BF16, kind="ExternalInput")
    if DEBUG:
        D["dbg_qT"] = nc.dram_tensor("dbg_qT", [128, 4096], BF16, kind="ExternalOutput")
        D["dbg_pe0"] = nc.dram_tensor("dbg_pe0", [128, 1024], BF16, kind="ExternalOutput")
        D["dbg_po0"] = nc.dram_tensor("dbg_po0", [128, 1024], BF16, kind="ExternalOutput")
        D["dbg_pse0"] = nc.dram_tensor("dbg_pse0", [65, 512], F32, kind="ExternalOutput")
        D["dbg_pso0"] = nc.dram_tensor("dbg_pso0", [65, 512], F32, kind="ExternalOutput")
        D["dbg_wv0"] = nc.dram_tensor("dbg_wv0", [128, 512], BF16, kind="ExternalOutput")
        D["dbg_rcp"] = nc.dram_tensor("dbg_rcp", [1, 1024], F32, kind="ExternalOutput")
        D["dbg_sce"] = nc.dram_tensor("dbg_sce", [64, 512], F32, kind="ExternalOutput")
    D["outT"] = nc.dram_tensor("outT", [NS, T], BF16, kind="ExternalOutput")
    D["keyT"] = nc.dram_tensor("keyT", [NS, T], BF16, kind="ExternalOutput")
    D["value"] = nc.dram_tensor("value", [T, NS], BF16, kind="ExternalOutput")

    with tile.TileContext(nc) as tc:
        with ExitStack() as ctx:
            _emit(ctx, tc, D)
    nc.compile()
    return nc


_NC_CACHE = None


def _get_nc():
    global _NC_CACHE
    if _NC_CACHE is None:
        _NC_CACHE = build()
    return _NC_CACHE


def prep_core_inputs(b, x, kv_cache, WqB, WkB, WvB, WoB, bqs, bop, bvr):
    xT = np.ascontiguousarray(x[b].T).reshape(8, 128, 512) \
        .transpose(1, 0, 2).reshape(128, 4096).astype(NPBF)
    kTc = np.ascontiguousarray(kv_cache[b, 0, 0].T).astype(NPBF)  # [NS, S]
    vjp = kv_cache[b, 0, 1].reshape(16, 128, NH, HD)  # [j, p, h, d]
    vh = vjp.transpose(2, 1, 0, 3)                    # [h, p, j, d]
    vb = np.ones((NC, 128, 16, 130), NPBF)
    vb[..., 0:64] = vh[0::2]
    vb[..., 65:129] = vh[1::2]
    return {
        "xT": xT, "kTc": kTc, "vb": vb.reshape(NC * 128, 2080),
        "Wq": WqB, "Wk": WkB, "Wv": WvB, "Wo": WoB,
        "bqs": bqs, "bop": bop, "bvr": bvr,
    }


def kernel(x, kv_cache, offset=0, Wq=None, bq=None, Wk=None, Wv=None, bv=None,
           Wo=None, bo=None, trace=False):
    global LAST_EXEC_NS, LAST_RESULTS
    x = np.asarray(x, np.float32)
    kv_cache = np.asarray(kv_cache, np.float32)
    Wq, bq, Wk, Wv, bv, Wo, bo = [np.asarray(a, np.float32)
                                  for a in (Wq, bq, Wk, Wv, bv, Wo, bo)]
    WqB, WkB, WvB, WoB = [w.astype(NPBF) for w in (Wq, Wk, Wv, Wo)]
    bqs = np.ascontiguousarray((bq * SCALE2).reshape(8, 128).T)
    bop = np.ascontiguousarray(bo.reshape(8, 128).T)
    bvr = bv[None, :].astype(NPBF)
    in_maps = [prep_core_inputs(b, x, kv_cache, WqB, WkB, WvB, WoB,
                                bqs, bop, bvr) for b in range(B)]
    nc = _get_nc()
    res = run_bass_kernel_spmd(nc, in_maps, core_ids=list(range(B)), trace=trace)
    LAST_EXEC_NS = res.exec_time_ns
    LAST_RESULTS = res
    out = np.stack([res.results[b]["outT"].astype(np.float32).T
                    for b in range(B)])
    key = np.stack([res.results[b]["keyT"].astype(np.float32).T
                    for b in range(B)])
    value = np.stack([res.results[b]["value"].astype(np.float32)
                      for b in range(B)])
    return (np.ascontiguousarray(out), np.ascontiguousarray(key),
            np.ascontiguousarray(value))
